# revision 8
# baseline (speedup 1.0000x reference)
"""Sliding-window causal self-attention (GQA + RoPE + tanh softcap) on 8 trn2 cores.

Sharding: core i = (b, g) with b = i // 4, g = i % 4.
Each core computes, for its batch b and kv-group g (4 q heads, 1 kv head):
    qkv projection (o-slice), RoPE, sliding-window attention, and the proj
    contribution of its o-slice:  out_partial[t, c] = sum_{o in slice} y[t,o] Wproj[c,o].
The host sums the 4 partials per batch (the "all-reduce after proj" done host-side).

All matmuls run as fp32r (full PE rate at N>=256 when warm). Layouts avoid
on-device transposes except v (PE-transpose via identity):
    xT      [C, T]   (host-transposed x[b])
    wqkvT   [C, 768] (host-transposed o-slice of Wqkv; o order: q0..q3, k, v)
    wprojT  [512, C] (host-transposed o-slice columns of Wproj)
    qT/kT   [d, t]   from  wT.T @ xT  (d on partitions -> scoresT = kT_tile.T @ qT)
    scoresT [j, i]   j (keys) on partitions, i (queries) on free axis
    P = exp(softcap(scores)) stays [j, i]; yT = v_tile.T @ P accumulates [d, i]
    rowsum via ones-matmul [1, i]; normalize = DVE recip + DRAM-bounce bcast.

Startup is HBM-bound (wqkv 6MB + x0 4MB stream at ~400GB/s): block-0 qkv is
emitted K-MAJOR over three open PSUM accumulators per phase (m 0-2, then
m 3-5) so each arriving (wq_k, x_k) chunk immediately yields 3 matmuls and
the PE tracks the DMA stream instead of stalling m-major.

Masked score tiles are NARROWED to their live column span (>=256 so fp32r
keeps 1 cyc/row): diag tile r spans [min(128r,256), 512), tail tile r spans
[0, max(128(r+1),256)). Scores/tanh/exp/PV/rowsum all honor the span, and
the 0/1 mask multiply only touches the 128-col staircase block (256 for
D3/T0 which carry a dead quarter). Window order puts a full-span tile first
so PSUM start=True covers every column.

Block pipeline: qkv m-tiles of block qt+1 and proj chunks of block qt-1 are
queued as small matmul units and drained INSIDE the attention j-loops, so
the PE stays busy while ACT works through tanh+exp latency. Block 3 (which
has no next-qkv) holds back qkv(3) m1/m2/m3 into its own attention phase,
with drain guards that force any unit producing qT/kT/v of head h to be
emitted before head h's scores.
"""

import math

import numpy as np

import concourse.bass as bass
import concourse.mybir as mybir
import concourse.tile as tile
from concourse.bass_utils import run_bass_kernel_spmd
from concourse.masks import make_identity

B, T, C = 2, 2048, 2048
N_HEAD, N_GROUPS, HEAD_SIZE = 16, 4, 128
SW = 1024
SOFTCAP = 50.0
QBLK = 512
NQB = T // QBLK          # 4 q-blocks
NKT = T // 128           # 16 key tiles
O_SLICE = 768            # 4 q heads + k + v  (128 each)
F32 = mybir.dt.float32
F32R = mybir.dt.float32r


def _window(qt):
    """Key-tile list for q-block qt: (kt, lo, hi, mask) with mask
    None | ('D', r) | ('T', r).  [lo, hi) is the live query-column span
    (clamped to >=256 wide for full-rate fp32r).  A full-span tile is
    always first so the PSUM start=True write covers all 512 columns."""
    wl = []
    for kt in range(max(0, 4 * qt - 4), 4 * qt):
        wl.append((kt, 0, 512, None))
    if qt >= 2:
        for r in range(4):
            wl.append((4 * qt - 8 + r, 0, max(128 * (r + 1), 256), ("T", r)))
    for r in range(4):
        wl.append((4 * qt + r, min(128 * r, 256), 512, ("D", r)))
    if qt == 0:
        # D0 has full span; it must lead for start=True coverage
        assert wl[0][3] == ("D", 0) and wl[0][1] == 0 and wl[0][2] == 512
    return wl


def _emit(tc, ctx):
    nc = tc.nc
    xT = nc.declare_dram_parameter("xT", [C, T], F32R, isOutput=False)
    wqkvT = nc.declare_dram_parameter("wqkvT", [C, O_SLICE], F32R, isOutput=False)
    wprojT = nc.declare_dram_parameter("wprojT", [512, C], F32R, isOutput=False)
    cosT = nc.declare_dram_parameter("cosT", [HEAD_SIZE, T], F32, isOutput=False)
    sinS = nc.declare_dram_parameter("sinS", [HEAD_SIZE, T], F32, isOutput=False)
    maskD = nc.declare_dram_parameter("maskD", [128, 256], F32R, isOutput=False)
    maskTl = nc.declare_dram_parameter("maskTl", [128, 256], F32R, isOutput=False)
    out = nc.declare_dram_parameter("out", [T, C], F32, isOutput=True)
    rscratch = nc.dram_tensor("rscratch", [NQB * 4, QBLK], F32)

    scale1 = 1.0 / (SOFTCAP * math.sqrt(HEAD_SIZE))

    consts = ctx.enter_context(tc.tile_pool(name="consts", bufs=1))
    xt_pool = ctx.enter_context(tc.tile_pool(name="xt", bufs=16))
    cs_pool = ctx.enter_context(tc.tile_pool(name="cs", bufs=2))
    rope_pool = ctx.enter_context(tc.tile_pool(name="rope", bufs=3))
    p_pool = ctx.enter_context(tc.tile_pool(name="pp", bufs=5))
    o_pool = ctx.enter_context(tc.tile_pool(name="op", bufs=3))
    r_pool = ctx.enter_context(tc.tile_pool(name="rp", bufs=2))
    ps = ctx.enter_context(tc.tile_pool(name="ps", space="PSUM", bufs=2))

    # ---- startup DMAs.  DMA *issues* cost ~650ns each on the issuing
    # queue, so the 10MB wq+x0 stream is split across two issue queues
    # (wq on GpSimd, x on Sync) to stay transfer-bound (~400GB/s), with
    # cos/sin/masks on the otherwise-idle Scalar queue. ----
    wq_sb = consts.tile([128, NKT, O_SLICE], F32R, name="wq_sb")
    x_tiles = {}  # (qt, k) -> tile
    for k in range(NKT):
        nc.gpsimd.dma_start(out=wq_sb[:, k, :], in_=wqkvT[k * 128:(k + 1) * 128, :])
        x_t = xt_pool.tile([128, QBLK], F32R, name=f"x_0_{k}", tag="xt")
        nc.sync.dma_start(out=x_t, in_=xT[k * 128:(k + 1) * 128, 0:QBLK])
        x_tiles[(0, k)] = x_t
    cs_tiles = {}
    cos_b = cs_pool.tile([128, QBLK], F32, name="cos_0", tag="cos")
    nc.scalar.dma_start(out=cos_b, in_=cosT[:, 0:QBLK])
    sin_b = cs_pool.tile([128, QBLK], F32, name="sin_0", tag="sin")
    nc.scalar.dma_start(out=sin_b, in_=sinS[:, 0:QBLK])
    cs_tiles[0] = (cos_b, sin_b)

    wp_sb = consts.tile([128, 4, C], F32R, name="wp_sb")  # loaded after A0
    mD_sb = consts.tile([128, 256], F32R, name="mD_sb")
    nc.scalar.dma_start(out=mD_sb, in_=maskD[:, :])
    mT_sb = consts.tile([128, 256], F32R, name="mT_sb")
    nc.scalar.dma_start(out=mT_sb, in_=maskTl[:, :])
    # all-ones column carved out of the diag mask (col 255 <-> x=127 >= all lj)
    ones_col = mD_sb[:, 255:256]             # [128, 1]
    ident = consts.tile([128, 128], F32, name="ident")
    make_identity(nc, ident)
    # warm the ACT exp/tanh table set during the startup DMAs (first real
    # tanh would otherwise pay the ~1.3us ACT_TABLE_LOAD mid-pipeline)
    warmup = consts.tile([1, 1], F32, name="warmup")
    nc.scalar.activation(warmup, ident[0:1, 0:1],
                         mybir.ActivationFunctionType.Tanh)

    # persistent activations (written per block, sub-tile deps handle reuse)
    kT_sb = consts.tile([128, T], F32R, name="kT_sb")          # roped k, [d, t]
    v_sb = consts.tile([128, NKT, 128], F32R, name="v_sb")     # [t128, kt, d]
    qT_sb = consts.tile([128, 4, QBLK], F32R, name="qT_sb")    # roped q, [d, h, i]
    y_tiles = {}  # qt -> [128, 4, QBLK] tile, bufs=2 across blocks

    def emit_loads(qt):
        t0 = qt * QBLK
        for k in range(NKT):
            x_t = xt_pool.tile([128, QBLK], F32R, name=f"x_{qt}_{k}", tag="xt")
            nc.sync.dma_start(out=x_t, in_=xT[k * 128:(k + 1) * 128, t0:t0 + QBLK])
            x_tiles[(qt, k)] = x_t
        cos_b = cs_pool.tile([128, QBLK], F32, name=f"cos_{qt}", tag="cos")
        nc.sync.dma_start(out=cos_b, in_=cosT[:, t0:t0 + QBLK])
        sin_b = cs_pool.tile([128, QBLK], F32, name=f"sin_{qt}", tag="sin")
        nc.sync.dma_start(out=sin_b, in_=sinS[:, t0:t0 + QBLK])
        cs_tiles[qt] = (cos_b, sin_b)

    def emit_rope(qt, m, psA):
        t0 = qt * QBLK
        cos_b, sin_b = cs_tiles[qt]
        # block 0: PSUM->SBUF copies on the idle ACT queue so the six
        # back-to-back startup ropes pipeline instead of serializing on DVE;
        # sin-multiply always on Pool to balance DVE
        copy = nc.scalar.copy if qt == 0 else nc.vector.tensor_copy
        if m < 5:
            # RoPE: dest = x*cos + rot(x)*sin_signed ; rot via DMA half-swap
            x_sb = rope_pool.tile([128, QBLK], F32, name=f"xsb_{qt}_{m}", tag="xsb")
            copy(x_sb, psA)
            rot = rope_pool.tile([128, QBLK], F32, name=f"rot_{qt}_{m}", tag="rot")
            nc.gpsimd.dma_start(out=rot[0:64, :], in_=x_sb[64:128, :])
            nc.gpsimd.dma_start(out=rot[64:128, :], in_=x_sb[0:64, :])
            dest = qT_sb[:, m, :] if m < 4 else kT_sb[:, t0:t0 + QBLK]
            nc.vector.tensor_mul(x_sb, x_sb, cos_b)
            nc.gpsimd.tensor_mul(rot, rot, sin_b)
            nc.vector.tensor_add(dest, x_sb, rot)
        else:
            # v: transpose [d, t] -> [t, d] tiles via PE (block 0's psT
            # borrows psA-tag banks freed by the m0/m1 ropes; later blocks
            # use psS as the scores stream frees those banks naturally)
            vt_sb = rope_pool.tile([128, QBLK], F32, name=f"vt_{qt}", tag="xsb")
            copy(vt_sb, psA)
            for i in range(4):
                psT = ps.tile([128, 128], F32, name=f"psT_{qt}_{i}",
                              tag="psA" if qt == 0 else "psS")
                nc.tensor.transpose(psT, vt_sb[:, i * 128:(i + 1) * 128], ident)
                nc.vector.tensor_copy(v_sb[:, qt * 4 + i, :], psT)

    def qkv_units(qt, m):
        """Fill units for one qkv m-tile: 8 x 2-matmul chunks + rope drain.
        Unit cost estimates are in ~us of PE time for the pop budget."""
        hold = {}

        def mk(i):
            def emit():
                if i == 0:
                    hold["psA"] = ps.tile([128, QBLK], F32,
                                          name=f"psA_{qt}_{m}", tag="psA")
                psA = hold["psA"]
                for k in (2 * i, 2 * i + 1):
                    nc.tensor.matmul(
                        psA,
                        wq_sb[:, k, m * 128:(m + 1) * 128],
                        x_tiles[(qt, k)],
                        start=(k == 0),
                        stop=(k == NKT - 1),
                    )
            return emit

        units = [(0.46, mk(i)) for i in range(8)]
        units.append((0.1, lambda: emit_rope(qt, m, hold["psA"])))
        return units

    # ---- startup: k-major qkv for block 0 (PE chases the DMA stream).
    # All six m-tiles accumulate simultaneously (6 of the 8 PSUM banks via
    # the psA/psS/psY tag pairs), so each arriving (wq_k, x_k) chunk yields
    # 6 matmuls immediately.  Ropes m2/m3 must precede m5: the v-transpose
    # PSUM tiles reuse the psS-tagged banks those accumulators hold. ----
    def startup_qkv():
        tags = ("psA", "psA", "psS", "psS", "psY", "psY")
        psQ = {m: ps.tile([128, QBLK], F32, name=f"psQ0_{m}", tag=tags[m],
                          bufs=3 if tags[m] == "psY" else 2)
               for m in range(6)}
        for k in range(NKT):
            for m in range(6):
                nc.tensor.matmul(
                    psQ[m],
                    wq_sb[:, k, m * 128:(m + 1) * 128],
                    x_tiles[(0, k)],
                    start=(k == 0),
                    stop=(k == NKT - 1),
                    skip_group_check=True,
                )
        # kT first (scores dep), then q0/q1 (freeing the psA banks the
        # v-transposes borrow), v, then q2/q3
        for m in (4, 0, 1, 5, 2, 3):
            emit_rope(0, m, psQ[m])

    from collections import deque, defaultdict

    fill_q = deque()          # (cost, emit, key)
    pending = defaultdict(int)

    def queue_units(units, key=None):
        for cost, emit in units:
            fill_q.append((cost, emit, key))
            if key is not None:
                pending[key] += 1

    def pop_one():
        cost, emit, key = fill_q.popleft()
        emit()
        if key is not None:
            pending[key] -= 1
        return cost

    def pop_fill(budget):
        """Emit queued qkv/proj matmul units worth ~budget us of PE time —
        keeps the PE fed while the attention stream waits on ACT latency."""
        spent = 0.0
        while fill_q and spent < budget:
            spent += pop_one()

    def drain_until(key):
        """Force-emit every queued unit up to and including those for key
        (FIFO) so tiles the next head reads are defined before use."""
        while pending.get(key, 0) > 0:
            pop_one()

    def head_mms(qt, h, wl):
        """Scores/tanh-exp/mask/pv/rowsum matmul stream for one head.
        Scores are emitted one j-tile ahead of the pv/rowsum consumers, and
        queued qkv/proj fill units are popped between them, so the PE stays
        busy while ACT works through the tanh+exp latency."""
        psY = ps.tile([128, QBLK], F32, name=f"psY_{qt}_{h}", tag="psY", bufs=3)
        psR = ps.tile([1, QBLK], F32, name=f"psR_{qt}_{h}", tag="psR", bufs=1)

        def emit_scores(idx):
            kt, lo, hi, mk = wl[idx]
            psS = ps.tile([128, QBLK], F32, name=f"psS_{qt}_{h}_{kt}", tag="psS")
            nc.tensor.matmul(
                psS[:, lo:hi], kT_sb[:, kt * 128:(kt + 1) * 128],
                qT_sb[:, h, lo:hi],
                start=True, stop=True,
            )
            p_t = p_pool.tile([128, QBLK], F32R, name=f"p_{qt}_{h}_{kt}", tag="p")
            nc.scalar.activation(
                p_t[:, lo:hi], psS[:, lo:hi],
                mybir.ActivationFunctionType.Tanh, scale=scale1
            )
            nc.scalar.activation(
                p_t[:, lo:hi], p_t[:, lo:hi],
                mybir.ActivationFunctionType.Exp, scale=SOFTCAP
            )
            if mk is not None:
                kind, r = mk
                if kind == "D":
                    if r == 3:  # cols [256,384) dead + staircase [384,512)
                        nc.vector.tensor_mul(p_t[:, 256:512], p_t[:, 256:512],
                                             mD_sb[:, 0:256])
                    else:       # staircase block [128r, 128r+128)
                        nc.vector.tensor_mul(
                            p_t[:, 128 * r:128 * r + 128],
                            p_t[:, 128 * r:128 * r + 128], mD_sb[:, 128:256])
                else:
                    if r == 0:  # staircase [0,128) + dead [128,256)
                        nc.vector.tensor_mul(p_t[:, 0:256], p_t[:, 0:256],
                                             mT_sb[:, 0:256])
                    else:       # staircase block [128r, 128r+128)
                        nc.vector.tensor_mul(
                            p_t[:, 128 * r:128 * r + 128],
                            p_t[:, 128 * r:128 * r + 128], mT_sb[:, 0:128])
            return p_t

        pts = {0: emit_scores(0)}
        for idx, (kt, lo, hi, mk) in enumerate(wl):
            if idx + 1 < len(wl):
                pts[idx + 1] = emit_scores(idx + 1)
            pop_fill(0.85)
            p_t = pts.pop(idx)
            first, last = idx == 0, idx == len(wl) - 1
            nc.tensor.matmul(
                psY[:, lo:hi], v_sb[:, kt, :], p_t[:, lo:hi],
                start=first, stop=last, skip_group_check=True,
            )
            nc.tensor.matmul(
                psR[:, lo:hi], ones_col, p_t[:, lo:hi],
                start=first, stop=last, skip_group_check=True,
            )
        return psY, psR

    def norm_head(qt, h, psY, psR):
        """Free both PSUM accumulators fast with copies, then run the
        reciprocal + partition-broadcast + multiply entirely on DVE/DMA —
        the PE never participates. recip runs on a [128,4] reshape (DVE
        recip is ~6 cyc/elem/lane; [1,512] would serialize 3.3us)."""
        rs = r_pool.tile([1, QBLK], F32, name=f"rs_{qt}_{h}", tag="rs")
        nc.vector.tensor_copy(rs, psR)
        yun = r_pool.tile([128, QBLK], F32, name=f"yun_{qt}_{h}", tag="yun")
        nc.vector.tensor_copy(yun, psY)
        rs128 = r_pool.tile([128, 4], F32, name=f"rs128_{qt}_{h}", tag="rs128")
        in_lin = bass.AP(tensor=rs.tensor, offset=rs.offset,
                         ap=[list(rs.ap[0]), [1, QBLK]])
        nc.gpsimd.dma_start(out=rs128, in_=in_lin)
        rr128 = r_pool.tile([128, 4], F32, name=f"rr128_{qt}_{h}", tag="rr128")
        nc.vector.reciprocal(rr128, rs128)
        # bounce through DRAM to broadcast across partitions (stride-0 DRAM
        # read on the way back — the standard partition-broadcast pattern)
        slot = rscratch[qt * 4 + h, :]
        nc.gpsimd.dma_start(out=slot, in_=rr128)
        rrf = r_pool.tile([128, QBLK], F32, name=f"rrf_{qt}_{h}", tag="rrf")
        bcast = bass.AP(tensor=slot.tensor, offset=slot.offset,
                        ap=[[0, 128], list(slot.ap[-1])])
        nc.gpsimd.dma_start(out=rrf, in_=bcast)
        nc.vector.tensor_mul(y_tiles[qt][:, h, :], yun, rrf)

    def proj_units(qt, mt):
        t0 = qt * QBLK

        def mk(cn):
            def emit():
                psP = ps.tile([128, 512], F32,
                              name=f"psP_{qt}_{mt}_{cn}", tag="psA")
                yt = y_tiles[qt]
                for kh in range(4):
                    nc.tensor.matmul(
                        psP,
                        yt[:, kh, mt * 128:(mt + 1) * 128],
                        wp_sb[:, kh, cn * 512:(cn + 1) * 512],
                        start=(kh == 0),
                        stop=(kh == 3),
                    )
                o_t = o_pool.tile([128, 512], F32,
                                  name=f"o_{qt}_{mt}_{cn}", tag="o")
                nc.vector.tensor_copy(o_t, psP)
                # stores issue on GpSimd: the Sync queue must stay free for
                # the next block's x loads (fills stall if those lag)
                nc.gpsimd.dma_start(
                    out=out[t0 + mt * 128: t0 + (mt + 1) * 128,
                            cn * 512:(cn + 1) * 512],
                    in_=o_t,
                )
            return emit

        return [(0.9, mk(cn)) for cn in range(4)]

    def emit_proj_chunk(qt, mt):
        for _, emit in proj_units(qt, mt):
            emit()

    # ---- interleaved pipeline with fine-grained fills ----
    startup_qkv()
    for qt in range(NQB):
        if qt + 1 < NQB:
            emit_loads(qt + 1)
        wl = _window(qt)
        y_tiles[qt] = consts.tile([128, 4, QBLK], F32R,
                                  name=f"yT_{qt}", tag="yT", bufs=2)
        for h in range(4):
            # qkv fills for the next block (block 3's m1/m2/m3 are held
            # back and queued during block 3's own attention)
            if qt + 1 < NQB:
                nxt = qt + 1
                if nxt < 3:
                    queue_units(qkv_units(nxt, (4, 5, 0, 1)[h]),
                                key=("q", nxt, (4, 5, 0, 1)[h]))
                elif h < 3:
                    queue_units(qkv_units(3, (4, 5, 0)[h]),
                                key=("q", 3, (4, 5, 0)[h]))
            else:
                if h < 3:
                    queue_units(qkv_units(3, h + 1), key=("q", 3, h + 1))
            # proj fills for the previous block (shifted one head later in
            # the last block so DVE runs the final normalizes promptly)
            if qt >= 1:
                if qt == NQB - 1:
                    if h >= 1:
                        queue_units(proj_units(qt - 1, h - 1))
                else:
                    queue_units(proj_units(qt - 1, h))
            # def-before-use: everything this head reads must be emitted
            if qt >= 1:
                drain_until(("q", qt, 4))
                drain_until(("q", qt, 5))
                drain_until(("q", qt, h))
            psY, psR = head_mms(qt, h, wl)
            norm_head(qt, h, psY, psR)
        if qt == 0:
            # cn-major chunks so proj(0, *, cn) deps resolve incrementally
            for cn in range(4):
                for kh in range(4):
                    nc.sync.dma_start(
                        out=wp_sb[:, kh, cn * 512:(cn + 1) * 512],
                        in_=wprojT[kh * 128:(kh + 1) * 128,
                                   cn * 512:(cn + 1) * 512])
        if qt + 1 < 3:
            queue_units(qkv_units(qt + 1, 2), key=("q", qt + 1, 2))
            queue_units(qkv_units(qt + 1, 3), key=("q", qt + 1, 3))
    queue_units(proj_units(NQB - 2, 3))
    while fill_q:
        pop_one()
    for mt in range(4):
        emit_proj_chunk(NQB - 1, mt)

_NC_CACHE = {}


def _build_nc():
    if "nc" not in _NC_CACHE:
        from contextlib import ExitStack

        from concourse import bacc

        nc = bacc.Bacc()
        with tile.TileContext(nc) as tc, ExitStack() as ctx:
            _emit(tc, ctx)
        nc.compile()
        _NC_CACHE["nc"] = nc
    return _NC_CACHE["nc"]


def _host_inputs(x, cos, sin, Wqkv, Wproj):
    """Build the 8 per-core input maps (sharding + layout transforms)."""
    cosT = np.ascontiguousarray(cos.T)                       # [128, T]
    sinT = sin.T
    sinS = np.concatenate([-sinT[:64], sinT[64:]], axis=0)   # sign-folded rotate-half
    sinS = np.ascontiguousarray(sinS)
    lj = np.arange(128)[:, None]
    xD = np.arange(256)[None, :] - 128                       # x = li - 128r in [-128,128)
    maskD = (xD >= lj).astype(np.float32)                    # diag: li - 128r >= lj
    xT_ = np.arange(256)[None, :]                            # x = li - 128r in [0,256)
    maskTl = (xT_ < lj).astype(np.float32)                   # tail: li - 128r <  lj
    q_sz = N_HEAD * HEAD_SIZE

    in_maps = []
    for core in range(8):
        b, g = core // 4, core % 4
        xTb = np.ascontiguousarray(x[b].T)                   # [C, T]
        wslice = np.concatenate(
            [
                Wqkv[512 * g: 512 * (g + 1)],                 # 4 q heads
                Wqkv[q_sz + 128 * g: q_sz + 128 * (g + 1)],   # k
                Wqkv[q_sz + 512 + 128 * g: q_sz + 512 + 128 * (g + 1)],  # v
            ],
            axis=0,
        )                                                     # [768, C]
        wqkvT = np.ascontiguousarray(wslice.T)                # [C, 768]
        wprojT = np.ascontiguousarray(Wproj[:, 512 * g: 512 * (g + 1)].T)  # [512, C]
        in_maps.append(
            {
                "xT": xTb,
                "wqkvT": wqkvT,
                "wprojT": wprojT,
                "cosT": cosT,
                "sinS": sinS,
                "maskD": maskD,
                "maskTl": maskTl,
            }
        )
    return in_maps


def kernel(x, cos, sin, Wqkv, Wproj, trace=False, tmpdir=None):
    x = np.asarray(x, dtype=np.float32)
    cos = np.asarray(cos, dtype=np.float32)
    sin = np.asarray(sin, dtype=np.float32)
    Wqkv = np.asarray(Wqkv, dtype=np.float32)
    Wproj = np.asarray(Wproj, dtype=np.float32)

    nc = _build_nc()
    in_maps = _host_inputs(x, cos, sin, Wqkv, Wproj)
    res = run_bass_kernel_spmd(nc, in_maps, list(range(8)), trace=trace, tmpdir=tmpdir)
    parts = [res.results[i]["out"] for i in range(8)]
    out = np.empty((B, T, C), dtype=np.float32)
    out[0] = parts[0] + parts[1] + parts[2] + parts[3]
    out[1] = parts[4] + parts[5] + parts[6] + parts[7]
    if trace:
        kernel.last_exec_time_ns = res.exec_time_ns
        kernel.last_results = res
    return out


# revision 21
# speedup vs baseline: 1.1037x; 1.1037x over previous
"""Sliding-window causal self-attention (GQA + RoPE + tanh softcap) on 8 trn2 cores.

Sharding: core i = (b, g) with b = i // 4, g = i % 4.
Each core computes, for its batch b and kv-group g (4 q heads, 1 kv head):
    qkv projection (o-slice), RoPE, sliding-window attention, and the proj
    contribution of its o-slice:  out_partial[t, c] = sum_{o in slice} y[t,o] Wproj[c,o].
The host sums the 4 partials per batch (the "all-reduce after proj" done host-side).

All matmuls run as fp32r (full PE rate at N>=256 when warm). Layouts avoid
on-device transposes except v (PE-transpose via identity):
    xT      [C, T]   (host-transposed x[b])
    wqkvT   [C, 768] (host-transposed o-slice of Wqkv; o order: q0..q3, k, v)
    wprojT  [512, C] (host-transposed o-slice columns of Wproj)
    qT/kT   [d, t]   from  wT.T @ xT  (d on partitions -> scoresT = kT_tile.T @ qT)
    scoresT [j, i]   j (keys) on partitions, i (queries) on free axis
    P = exp(softcap(scores)) stays [j, i]; yT = v_tile.T @ P accumulates [d, i]
    rowsum via ones-matmul into psB row 0; normalize = [128,4]-reshaped DVE
    recip, ones-row PE matmul broadcasting 1/rowsum across psB, DVE mult.

Startup is HBM-bound (wqkv 6MB + x0 4MB stream at ~400GB/s): block-0 qkv is
emitted K-MAJOR over three open PSUM accumulators per phase (m 0-2, then
m 3-5) so each arriving (wq_k, x_k) chunk immediately yields 3 matmuls and
the PE tracks the DMA stream instead of stalling m-major.

Masked score tiles are NARROWED to their live column span (>=256 so fp32r
keeps 1 cyc/row): diag tile r spans [min(128r,256), 512), tail tile r spans
[0, max(128(r+1),256)). Scores/tanh/exp/PV/rowsum all honor the span, and
the 0/1 mask multiply only touches the 128-col staircase block (256 for
D3/T0 which carry a dead quarter). Window order puts a full-span tile first
so PSUM start=True covers every column.

Block pipeline: qkv m-tiles of block qt+1 and proj chunks of block qt-1 are
queued as small matmul units and drained INSIDE the attention j-loops, so
the PE stays busy while ACT works through tanh+exp latency. Block 3 (which
has no next-qkv) holds back qkv(3) m1/m2/m3 into its own attention phase,
with drain guards that force any unit producing qT/kT/v of head h to be
emitted before head h's scores.
"""

import math

import numpy as np

import concourse.bass as bass
import concourse.mybir as mybir
import concourse.tile as tile
from concourse.bass_utils import run_bass_kernel_spmd
from concourse.masks import make_identity

B, T, C = 2, 2048, 2048
N_HEAD, N_GROUPS, HEAD_SIZE = 16, 4, 128
SW = 1024
SOFTCAP = 50.0
QBLK = 512
NQB = T // QBLK          # 4 q-blocks
NKT = T // 128           # 16 key tiles
O_SLICE = 768            # 4 q heads + k + v  (128 each)
F32 = mybir.dt.float32
F32R = mybir.dt.float32r


def _window(qt):
    """Key-tile list for q-block qt: (kt, lo, hi, mask) with mask
    None | ('D', r) | ('T', r).  [lo, hi) is the live query-column span
    (clamped to >=256 wide for full-rate fp32r).  A full-span tile is
    always first so the PSUM start=True write covers all 512 columns."""
    wl = []
    for kt in range(max(0, 4 * qt - 4), 4 * qt):
        wl.append((kt, 0, 512, None))
    if qt >= 2:
        for r in range(4):
            wl.append((4 * qt - 8 + r, 0, max(128 * (r + 1), 256), ("T", r)))
    for r in range(4):
        wl.append((4 * qt + r, min(128 * r, 256), 512, ("D", r)))
    if qt == 0:
        # D0 has full span; it must lead for start=True coverage
        assert wl[0][3] == ("D", 0) and wl[0][1] == 0 and wl[0][2] == 512
    return wl


def _emit(tc, ctx):
    nc = tc.nc
    xT = nc.declare_dram_parameter("xT", [C, T], F32R, isOutput=False)
    wqkvT = nc.declare_dram_parameter("wqkvT", [C, O_SLICE], F32R, isOutput=False)
    wprojT = nc.declare_dram_parameter("wprojT", [512, C], F32R, isOutput=False)
    cosT = nc.declare_dram_parameter("cosT", [HEAD_SIZE, T], F32, isOutput=False)
    sinS = nc.declare_dram_parameter("sinS", [HEAD_SIZE, T], F32, isOutput=False)
    maskD = nc.declare_dram_parameter("maskD", [128, 256], F32R, isOutput=False)
    maskTl = nc.declare_dram_parameter("maskTl", [128, 256], F32R, isOutput=False)
    out = nc.declare_dram_parameter("out", [T, C], F32, isOutput=True)

    scale1 = 1.0 / (SOFTCAP * math.sqrt(HEAD_SIZE))

    consts = ctx.enter_context(tc.tile_pool(name="consts", bufs=1))
    xt_pool = ctx.enter_context(tc.tile_pool(name="xt", bufs=16))
    cs_pool = ctx.enter_context(tc.tile_pool(name="cs", bufs=2))
    rope_pool = ctx.enter_context(tc.tile_pool(name="rope", bufs=3))
    p_pool = ctx.enter_context(tc.tile_pool(name="pp", bufs=5))
    o_pool = ctx.enter_context(tc.tile_pool(name="op", bufs=3))
    r_pool = ctx.enter_context(tc.tile_pool(name="rp", bufs=2))
    ps = ctx.enter_context(tc.tile_pool(name="ps", space="PSUM", bufs=2))

    # ---- startup DMAs: interleaved (wq_k, x_k) pairs on the Sync queue.
    # One DMA instruction rides one DMA engine, so many mid-size DMAs beat
    # few large ones; the stream is transfer-bound at ~400GB/s. ----
    wq_sb = consts.tile([128, NKT, O_SLICE], F32R, name="wq_sb")
    x_tiles = {}  # (qt, k) -> tile
    for k in range(NKT):
        nc.sync.dma_start(out=wq_sb[:, k, :], in_=wqkvT[k * 128:(k + 1) * 128, :])
        x_t = xt_pool.tile([128, QBLK], F32R, name=f"x_0_{k}", tag="xt")
        nc.sync.dma_start(out=x_t, in_=xT[k * 128:(k + 1) * 128, 0:QBLK])
        x_tiles[(0, k)] = x_t
    cs_tiles = {}
    cos_b = cs_pool.tile([128, QBLK], F32, name="cos_0", tag="cos")
    nc.sync.dma_start(out=cos_b, in_=cosT[:, 0:QBLK])
    sin_b = cs_pool.tile([128, QBLK], F32, name="sin_0", tag="sin")
    nc.sync.dma_start(out=sin_b, in_=sinS[:, 0:QBLK])
    cs_tiles[0] = (cos_b, sin_b)

    wp_sb = consts.tile([128, 4, C], F32R, name="wp_sb")  # loaded after A0
    mD_sb = consts.tile([128, 256], F32R, name="mD_sb")
    nc.sync.dma_start(out=mD_sb, in_=maskD[:, :])
    mT_sb = consts.tile([128, 256], F32R, name="mT_sb")
    nc.sync.dma_start(out=mT_sb, in_=maskTl[:, :])
    # all-ones views carved out of the diag mask: col 255 <-> x=127 >= all
    # lj; row 0 cols 128:256 <-> x in [0,128) >= lj=0
    ones_col = mD_sb[:, 255:256]             # [128, 1]
    ones_row = mD_sb[0:1, 128:256]           # [1, 128]
    ident = consts.tile([128, 128], F32, name="ident")
    make_identity(nc, ident)
    # warm the ACT exp/tanh table set during the startup DMAs (first real
    # tanh would otherwise pay the ~1.3us ACT_TABLE_LOAD mid-pipeline)
    warmup = consts.tile([1, 1], F32, name="warmup")
    nc.scalar.activation(warmup, ident[0:1, 0:1],
                         mybir.ActivationFunctionType.Tanh)

    # persistent activations (written per block, sub-tile deps handle reuse)
    kT_sb = consts.tile([128, T], F32R, name="kT_sb")          # roped k, [d, t]
    v_sb = consts.tile([128, NKT, 128], F32R, name="v_sb")     # [t128, kt, d]
    qT_sb = consts.tile([128, 4, QBLK], F32R, name="qT_sb")    # roped q, [d, h, i]
    y_tiles = {}  # qt -> [128, 4, QBLK] tile, bufs=2 across blocks

    def emit_loads(qt):
        t0 = qt * QBLK
        for k in range(NKT):
            x_t = xt_pool.tile([128, QBLK], F32R, name=f"x_{qt}_{k}", tag="xt")
            nc.sync.dma_start(out=x_t, in_=xT[k * 128:(k + 1) * 128, t0:t0 + QBLK])
            x_tiles[(qt, k)] = x_t
        cos_b = cs_pool.tile([128, QBLK], F32, name=f"cos_{qt}", tag="cos")
        nc.sync.dma_start(out=cos_b, in_=cosT[:, t0:t0 + QBLK])
        sin_b = cs_pool.tile([128, QBLK], F32, name=f"sin_{qt}", tag="sin")
        nc.sync.dma_start(out=sin_b, in_=sinS[:, t0:t0 + QBLK])
        cs_tiles[qt] = (cos_b, sin_b)

    def emit_rope(qt, m, psA):
        t0 = qt * QBLK
        cos_b, sin_b = cs_tiles[qt]
        # block 0: PSUM->SBUF copies on the idle ACT queue so the six
        # back-to-back startup ropes pipeline instead of serializing on DVE;
        # sin-multiply always on Pool to balance DVE
        copy = nc.scalar.copy if qt == 0 else nc.vector.tensor_copy
        if m < 5:
            # RoPE: dest = x*cos + rot(x)*sin_signed ; rot via DMA half-swap
            x_sb = rope_pool.tile([128, QBLK], F32, name=f"xsb_{qt}_{m}", tag="xsb")
            copy(x_sb, psA)
            rot = rope_pool.tile([128, QBLK], F32, name=f"rot_{qt}_{m}", tag="rot")
            nc.gpsimd.dma_start(out=rot[0:64, :], in_=x_sb[64:128, :])
            nc.gpsimd.dma_start(out=rot[64:128, :], in_=x_sb[0:64, :])
            dest = qT_sb[:, m, :] if m < 4 else kT_sb[:, t0:t0 + QBLK]
            nc.vector.tensor_mul(x_sb, x_sb, cos_b)
            nc.vector.tensor_mul(rot, rot, sin_b)
            nc.vector.tensor_add(dest, x_sb, rot)
        else:
            # v: transpose [d, t] -> [t, d] tiles via PE (block 0's psT
            # borrows psA-tag banks freed by the m0/m1 ropes; later blocks
            # use psS as the scores stream frees those banks naturally)
            vt_sb = rope_pool.tile([128, QBLK], F32, name=f"vt_{qt}", tag="xsb")
            copy(vt_sb, psA)
            for i in range(4):
                psT = ps.tile([128, 128], F32, name=f"psT_{qt}_{i}",
                              tag="psA" if qt == 0 else "psS")
                nc.tensor.transpose(psT, vt_sb[:, i * 128:(i + 1) * 128], ident)
                nc.vector.tensor_copy(v_sb[:, qt * 4 + i, :], psT)

    def qkv_units(qt, m):
        """Fill units for one qkv m-tile: 8 x 2-matmul chunks + rope drain.
        Unit cost estimates are in ~us of PE time for the pop budget."""
        hold = {}

        def mk(i):
            def emit():
                if i == 0:
                    hold["psA"] = ps.tile([128, QBLK], F32,
                                          name=f"psA_{qt}_{m}", tag="psA")
                psA = hold["psA"]
                for k in (2 * i, 2 * i + 1):
                    nc.tensor.matmul(
                        psA,
                        wq_sb[:, k, m * 128:(m + 1) * 128],
                        x_tiles[(qt, k)],
                        start=(k == 0),
                        stop=(k == NKT - 1),
                    )
            return emit

        units = [(0.46, mk(i)) for i in range(8)]
        units.append((0.1, lambda: emit_rope(qt, m, hold["psA"])))
        return units

    # ---- startup: k-major qkv for block 0 (PE chases the DMA stream).
    # All six m-tiles accumulate simultaneously (6 of the 8 PSUM banks via
    # the psA/psS/psY tag pairs), so each arriving (wq_k, x_k) chunk yields
    # 6 matmuls immediately.  Ropes m2/m3 must precede m5: the v-transpose
    # PSUM tiles reuse the psS-tagged banks those accumulators hold. ----
    def startup_qkv():
        tags = ("psA", "psA", "psS", "psS", "psY", "psY")
        psQ = {m: ps.tile([128, QBLK], F32, name=f"psQ0_{m}", tag=tags[m],
                          bufs=2)
               for m in range(6)}
        for k in range(NKT):
            for m in range(6):
                nc.tensor.matmul(
                    psQ[m],
                    wq_sb[:, k, m * 128:(m + 1) * 128],
                    x_tiles[(0, k)],
                    start=(k == 0),
                    stop=(k == NKT - 1),
                    skip_group_check=True,
                )
        # kT first (scores dep), then q0/q1 (freeing the psA banks the
        # v-transposes borrow), v, then q2/q3
        for m in (4, 0, 1, 5, 2, 3):
            emit_rope(0, m, psQ[m])

    from collections import deque, defaultdict

    fill_q = deque()          # (cost, emit, key)
    pending = defaultdict(int)

    def queue_units(units, key=None):
        for cost, emit in units:
            fill_q.append((cost, emit, key))
            if key is not None:
                pending[key] += 1

    def pop_one():
        cost, emit, key = fill_q.popleft()
        emit()
        if key is not None:
            pending[key] -= 1
        return cost

    def pop_fill(budget):
        """Emit queued qkv/proj matmul units worth ~budget us of PE time —
        keeps the PE fed while the attention stream waits on ACT latency."""
        spent = 0.0
        while fill_q and spent < budget:
            spent += pop_one()

    def drain_until(key):
        """Force-emit every queued unit up to and including those for key
        (FIFO) so tiles the next head reads are defined before use."""
        while pending.get(key, 0) > 0:
            pop_one()

    def head_mms(qt, h, wl):
        """Scores/tanh-exp/mask/pv/rowsum matmul stream for one head.
        Scores are emitted one j-tile ahead of the pv/rowsum consumers, and
        queued qkv/proj fill units are popped between them, so the PE stays
        busy while ACT works through the tanh+exp latency."""
        psY = ps.tile([128, QBLK], F32, name=f"psY_{qt}_{h}", tag="psY", bufs=2)
        psB = ps.tile([128, QBLK], F32, name=f"psB_{qt}_{h}", tag="psB", bufs=2)

        def emit_scores(idx):
            kt, lo, hi, mk = wl[idx]
            psS = ps.tile([128, QBLK], F32, name=f"psS_{qt}_{h}_{kt}", tag="psS")
            nc.tensor.matmul(
                psS[:, lo:hi], kT_sb[:, kt * 128:(kt + 1) * 128],
                qT_sb[:, h, lo:hi],
                start=True, stop=True,
            )
            p_t = p_pool.tile([128, QBLK], F32R, name=f"p_{qt}_{h}_{kt}", tag="p")
            nc.scalar.activation(
                p_t[:, lo:hi], psS[:, lo:hi],
                mybir.ActivationFunctionType.Tanh, scale=scale1
            )
            nc.scalar.activation(
                p_t[:, lo:hi], p_t[:, lo:hi],
                mybir.ActivationFunctionType.Exp, scale=SOFTCAP
            )
            if mk is not None:
                kind, r = mk
                if kind == "D":
                    if r == 3:  # cols [256,384) dead + staircase [384,512)
                        nc.vector.tensor_mul(p_t[:, 256:512], p_t[:, 256:512],
                                             mD_sb[:, 0:256])
                    else:       # staircase block [128r, 128r+128)
                        nc.vector.tensor_mul(
                            p_t[:, 128 * r:128 * r + 128],
                            p_t[:, 128 * r:128 * r + 128], mD_sb[:, 128:256])
                else:
                    if r == 0:  # staircase [0,128) + dead [128,256)
                        nc.vector.tensor_mul(p_t[:, 0:256], p_t[:, 0:256],
                                             mT_sb[:, 0:256])
                    else:       # staircase block [128r, 128r+128)
                        nc.vector.tensor_mul(
                            p_t[:, 128 * r:128 * r + 128],
                            p_t[:, 128 * r:128 * r + 128], mT_sb[:, 0:128])
            return p_t

        pts = {0: emit_scores(0)}
        for idx, (kt, lo, hi, mk) in enumerate(wl):
            if idx + 1 < len(wl):
                pts[idx + 1] = emit_scores(idx + 1)
            pop_fill(0.85)
            p_t = pts.pop(idx)
            first, last = idx == 0, idx == len(wl) - 1
            nc.tensor.matmul(
                psY[:, lo:hi], v_sb[:, kt, :], p_t[:, lo:hi],
                start=first, stop=last, skip_group_check=True,
            )
            nc.tensor.matmul(
                psB[0:1, lo:hi], ones_col, p_t[:, lo:hi],
                start=first, stop=last, skip_group_check=True,
            )
        return psY, psB

    def norm_head(qt, h, psY, psB):
        """Stage A: free both PSUM accumulators fast with copies, then build
        1/rowsum as [1,512] via a [128,4] reshape (DVE recip is ~6
        cyc/elem/lane; [1,512] would serialize 3.3us).  The returned
        finisher broadcasts it across partitions with a ones-column matmul
        into psB (overwriting the rowsum row) and multiplies — it is
        deferred into the NEXT head's j-loop so the PE never waits on the
        reshape/recip chain."""
        rs = r_pool.tile([1, QBLK], F32, name=f"rs_{qt}_{h}", tag="rs")
        nc.vector.tensor_copy(rs, psB[0:1, :])
        yun = r_pool.tile([128, QBLK], F32, name=f"yun_{qt}_{h}", tag="yun")
        nc.vector.tensor_copy(yun, psY)
        rs128 = r_pool.tile([128, 4], F32, name=f"rs128_{qt}_{h}", tag="rs128")
        in_lin = bass.AP(tensor=rs.tensor, offset=rs.offset,
                         ap=[list(rs.ap[0]), [1, QBLK]])
        nc.gpsimd.dma_start(out=rs128, in_=in_lin)
        rr128 = r_pool.tile([128, 4], F32, name=f"rr128_{qt}_{h}", tag="rr128")
        nc.vector.reciprocal(rr128, rs128)
        rr_row = r_pool.tile([1, QBLK], F32R, name=f"rrw_{qt}_{h}", tag="rrw")
        row_view = bass.AP(tensor=rr_row.tensor, offset=rr_row.offset,
                           ap=[list(rr_row.ap[0]), [1, QBLK]])
        nc.gpsimd.dma_start(out=row_view, in_=rr128)

        def finish():
            nc.tensor.matmul(psB, ones_row, rr_row, start=True, stop=True,
                             skip_group_check=True)
            nc.vector.tensor_mul(y_tiles[qt][:, h, :], yun, psB)
        return finish

    def proj_units(qt, mt):
        t0 = qt * QBLK

        def mk(cn):
            def emit():
                psP = ps.tile([128, 512], F32,
                              name=f"psP_{qt}_{mt}_{cn}", tag="psA")
                yt = y_tiles[qt]
                for kh in range(4):
                    nc.tensor.matmul(
                        psP,
                        yt[:, kh, mt * 128:(mt + 1) * 128],
                        wp_sb[:, kh, cn * 512:(cn + 1) * 512],
                        start=(kh == 0),
                        stop=(kh == 3),
                    )
                o_t = o_pool.tile([128, 512], F32,
                                  name=f"o_{qt}_{mt}_{cn}", tag="o")
                nc.vector.tensor_copy(o_t, psP)
                nc.sync.dma_start(
                    out=out[t0 + mt * 128: t0 + (mt + 1) * 128,
                            cn * 512:(cn + 1) * 512],
                    in_=o_t,
                )
            return emit

        return [(0.9, mk(cn)) for cn in range(4)]

    def emit_proj_chunk(qt, mt):
        for _, emit in proj_units(qt, mt):
            emit()

    finz = {}  # (qt, h) -> deferred normalize finisher

    def push_fin(qt, h):
        f = finz.pop((qt, h), None)
        if f is not None:
            fill_q.appendleft((0.25, f, None))

    # ---- interleaved pipeline with fine-grained fills ----
    startup_qkv()
    for qt in range(NQB):
        if qt + 1 < NQB:
            emit_loads(qt + 1)
        wl = _window(qt)
        y_tiles[qt] = consts.tile([128, 4, QBLK], F32R,
                                  name=f"yT_{qt}", tag="yT", bufs=2)
        for h in range(4):
            # finishers ride the FRONT of the fill queue two heads after
            # their stage A, so the PE never waits on the recip chain but
            # y[h] is always emitted before any proj fill that reads it
            if h == 0 and qt >= 1:
                push_fin(qt - 1, 3)
                push_fin(qt - 1, 2)
            elif h >= 2:
                push_fin(qt, h - 2)
            # qkv fills for the next block (block 3's m1/m2/m3 are held
            # back and queued during block 3's own attention)
            if qt + 1 < NQB:
                nxt = qt + 1
                if nxt < 3:
                    queue_units(qkv_units(nxt, (4, 5, 0, 1)[h]),
                                key=("q", nxt, (4, 5, 0, 1)[h]))
                elif h < 3:
                    queue_units(qkv_units(3, (4, 5, 0)[h]),
                                key=("q", 3, (4, 5, 0)[h]))
            else:
                if h < 3:
                    queue_units(qkv_units(3, h + 1), key=("q", 3, h + 1))
            # proj fills for the previous block (shifted one head later in
            # the last block so DVE runs the final normalizes promptly)
            if qt >= 1:
                if qt == NQB - 1:
                    if h >= 1:
                        queue_units(proj_units(qt - 1, h - 1))
                else:
                    queue_units(proj_units(qt - 1, h))
            # def-before-use: everything this head reads must be emitted
            if qt >= 1:
                drain_until(("q", qt, 4))
                drain_until(("q", qt, 5))
                drain_until(("q", qt, h))
            psY, psB = head_mms(qt, h, wl)
            finz[(qt, h)] = norm_head(qt, h, psY, psB)
        if qt == 0:
            # cn-major chunks so proj(0, *, cn) deps resolve incrementally
            for cn in range(4):
                for kh in range(4):
                    nc.sync.dma_start(
                        out=wp_sb[:, kh, cn * 512:(cn + 1) * 512],
                        in_=wprojT[kh * 128:(kh + 1) * 128,
                                   cn * 512:(cn + 1) * 512])
        if qt + 1 < 3:
            queue_units(qkv_units(qt + 1, 2), key=("q", qt + 1, 2))
            queue_units(qkv_units(qt + 1, 3), key=("q", qt + 1, 3))
    queue_units(proj_units(NQB - 2, 3))
    while fill_q:
        pop_one()
    finz.pop((NQB - 1, 2))()
    finz.pop((NQB - 1, 3))()
    for mt in range(4):
        emit_proj_chunk(NQB - 1, mt)

_NC_CACHE = {}


def _build_nc():
    if "nc" not in _NC_CACHE:
        from contextlib import ExitStack

        from concourse import bacc

        nc = bacc.Bacc()
        with tile.TileContext(nc) as tc, ExitStack() as ctx:
            _emit(tc, ctx)
        nc.compile()
        _NC_CACHE["nc"] = nc
    return _NC_CACHE["nc"]


def _host_inputs(x, cos, sin, Wqkv, Wproj):
    """Build the 8 per-core input maps (sharding + layout transforms)."""
    cosT = np.ascontiguousarray(cos.T)                       # [128, T]
    sinT = sin.T
    sinS = np.concatenate([-sinT[:64], sinT[64:]], axis=0)   # sign-folded rotate-half
    sinS = np.ascontiguousarray(sinS)
    lj = np.arange(128)[:, None]
    xD = np.arange(256)[None, :] - 128                       # x = li - 128r in [-128,128)
    maskD = (xD >= lj).astype(np.float32)                    # diag: li - 128r >= lj
    xT_ = np.arange(256)[None, :]                            # x = li - 128r in [0,256)
    maskTl = (xT_ < lj).astype(np.float32)                   # tail: li - 128r <  lj
    q_sz = N_HEAD * HEAD_SIZE

    in_maps = []
    for core in range(8):
        b, g = core // 4, core % 4
        xTb = np.ascontiguousarray(x[b].T)                   # [C, T]
        wslice = np.concatenate(
            [
                Wqkv[512 * g: 512 * (g + 1)],                 # 4 q heads
                Wqkv[q_sz + 128 * g: q_sz + 128 * (g + 1)],   # k
                Wqkv[q_sz + 512 + 128 * g: q_sz + 512 + 128 * (g + 1)],  # v
            ],
            axis=0,
        )                                                     # [768, C]
        wqkvT = np.ascontiguousarray(wslice.T)                # [C, 768]
        wprojT = np.ascontiguousarray(Wproj[:, 512 * g: 512 * (g + 1)].T)  # [512, C]
        in_maps.append(
            {
                "xT": xTb,
                "wqkvT": wqkvT,
                "wprojT": wprojT,
                "cosT": cosT,
                "sinS": sinS,
                "maskD": maskD,
                "maskTl": maskTl,
            }
        )
    return in_maps


def kernel(x, cos, sin, Wqkv, Wproj, trace=False, tmpdir=None):
    x = np.asarray(x, dtype=np.float32)
    cos = np.asarray(cos, dtype=np.float32)
    sin = np.asarray(sin, dtype=np.float32)
    Wqkv = np.asarray(Wqkv, dtype=np.float32)
    Wproj = np.asarray(Wproj, dtype=np.float32)

    nc = _build_nc()
    in_maps = _host_inputs(x, cos, sin, Wqkv, Wproj)
    res = run_bass_kernel_spmd(nc, in_maps, list(range(8)), trace=trace, tmpdir=tmpdir)
    parts = [res.results[i]["out"] for i in range(8)]
    out = np.empty((B, T, C), dtype=np.float32)
    out[0] = parts[0] + parts[1] + parts[2] + parts[3]
    out[1] = parts[4] + parts[5] + parts[6] + parts[7]
    if trace:
        kernel.last_exec_time_ns = res.exec_time_ns
        kernel.last_results = res
    return out


# revision 27
# speedup vs baseline: 1.2507x; 1.1332x over previous
"""Sliding-window causal self-attention (GQA + RoPE + tanh softcap) on 8 trn2 cores.

Sharding: core i = (b, g) with b = i // 4, g = i % 4.
Each core computes, for its batch b and kv-group g (4 q heads, 1 kv head):
    qkv projection (o-slice), RoPE, sliding-window attention, and the proj
    contribution of its o-slice:  out_partial[t, c] = sum_{o in slice} y[t,o] Wproj[c,o].
The host sums the 4 partials per batch (the "all-reduce after proj" done host-side).

All matmuls run as fp32r (full PE rate at N>=256 when warm). Layouts avoid
on-device transposes except v (PE-transpose via identity):
    xT      [C, T]   (host-transposed x[b])
    wqkvT   [C, 768] (host-transposed o-slice of Wqkv; o order: q0..q3, k, v)
    wprojT  [512, C] (host-transposed o-slice columns of Wproj)
    qT/kT   [d, t]   from  wT.T @ xT  (d on partitions -> scoresT = kT_tile.T @ qT)
    scoresT [j, i]   j (keys) on partitions, i (queries) on free axis
    P = exp(softcap(scores)) stays [j, i]; yT = v_tile.T @ P accumulates [d, i]
    rowsum via ones-matmul into psB row 0; normalize = [128,4]-reshaped DVE
    recip, ones-row PE matmul broadcasting 1/rowsum across psB, DVE mult.

Startup is HBM-bound (wqkv 6MB + x0 4MB stream at ~400GB/s): block-0 qkv is
emitted K-MAJOR over three open PSUM accumulators per phase (m 0-2, then
m 3-5) so each arriving (wq_k, x_k) chunk immediately yields 3 matmuls and
the PE tracks the DMA stream instead of stalling m-major.

Masked score tiles are NARROWED to their live column span (>=256 so fp32r
keeps 1 cyc/row): diag tile r spans [min(128r,256), 512), tail tile r spans
[0, max(128(r+1),256)). Scores/tanh/exp/PV/rowsum all honor the span, and
the 0/1 mask multiply only touches the 128-col staircase block (256 for
D3/T0 which carry a dead quarter). Window order puts a full-span tile first
so PSUM start=True covers every column.

Block pipeline: qkv m-tiles of block qt+1 and proj chunks of block qt-1 are
queued as small matmul units and drained INSIDE the attention j-loops, so
the PE stays busy while ACT works through tanh+exp latency. Block 3 (which
has no next-qkv) holds back qkv(3) m1/m2/m3 into its own attention phase,
with drain guards that force any unit producing qT/kT/v of head h to be
emitted before head h's scores.
"""

import math

import ml_dtypes
import numpy as np

import concourse.bass as bass
import concourse.mybir as mybir
import concourse.tile as tile
from concourse.bass_utils import run_bass_kernel_spmd
from concourse.masks import make_identity

B, T, C = 2, 2048, 2048
N_HEAD, N_GROUPS, HEAD_SIZE = 16, 4, 128
SW = 1024
SOFTCAP = 50.0
QBLK = 512
NQB = T // QBLK          # 4 q-blocks
NKT = T // 128           # 16 key tiles
O_SLICE = 768            # 4 q heads + k + v  (128 each)
F32 = mybir.dt.float32
F32R = mybir.dt.float32r
BF16 = mybir.dt.bfloat16


def _window(qt):
    """Key-tile list for q-block qt: (kt, lo, hi, mask) with mask
    None | ('D', r) | ('T', r).  [lo, hi) is the live query-column span
    (clamped to >=256 wide for full-rate fp32r).  A full-span tile is
    always first so the PSUM start=True write covers all 512 columns."""
    wl = []
    for kt in range(max(0, 4 * qt - 4), 4 * qt):
        wl.append((kt, 0, 512, None))
    if qt >= 2:
        for r in range(4):
            wl.append((4 * qt - 8 + r, 0, 128 * (r + 1), ("T", r)))
    for r in range(4):
        wl.append((4 * qt + r, 128 * r, 512, ("D", r)))
    if qt == 0:
        # D0 has full span; it must lead for start=True coverage
        assert wl[0][3] == ("D", 0) and wl[0][1] == 0 and wl[0][2] == 512
    return wl


def _emit(tc, ctx):
    nc = tc.nc
    xT = nc.declare_dram_parameter("xT", [C, T], BF16, isOutput=False)
    wqkvT = nc.declare_dram_parameter("wqkvT", [C, O_SLICE], BF16, isOutput=False)
    wprojT = nc.declare_dram_parameter("wprojT", [512, C], BF16, isOutput=False)
    cosT = nc.declare_dram_parameter("cosT", [HEAD_SIZE, T], BF16, isOutput=False)
    sinS = nc.declare_dram_parameter("sinS", [HEAD_SIZE, T], BF16, isOutput=False)
    maskD = nc.declare_dram_parameter("maskD", [128, 128], BF16, isOutput=False)
    maskTl = nc.declare_dram_parameter("maskTl", [128, 128], BF16, isOutput=False)
    out = nc.declare_dram_parameter("out", [T, C], F32, isOutput=True)

    scale1 = 1.0 / (SOFTCAP * math.sqrt(HEAD_SIZE))

    consts = ctx.enter_context(tc.tile_pool(name="consts", bufs=1))
    xt_pool = ctx.enter_context(tc.tile_pool(name="xt", bufs=16))
    cs_pool = ctx.enter_context(tc.tile_pool(name="cs", bufs=2))
    rope_pool = ctx.enter_context(tc.tile_pool(name="rope", bufs=3))
    p_pool = ctx.enter_context(tc.tile_pool(name="pp", bufs=5))
    o_pool = ctx.enter_context(tc.tile_pool(name="op", bufs=3))
    r_pool = ctx.enter_context(tc.tile_pool(name="rp", bufs=2))
    ps = ctx.enter_context(tc.tile_pool(name="ps", space="PSUM", bufs=2))

    # ---- small constants (cos/sin/masks) go FIRST on the idle GpSimd
    # queue — behind the Sync stream they'd land ~45us in and stall the
    # block-0 ropes.  The big wq+x0 stream rides Sync as interleaved
    # (wq_k, x_k) pairs: one DMA instruction rides one DMA engine, so many
    # mid-size DMAs beat few large ones (transfer-bound ~400GB/s). ----
    cs_tiles = {}
    cos_b = cs_pool.tile([128, QBLK], BF16, name="cos_0", tag="cos")
    nc.gpsimd.dma_start(out=cos_b, in_=cosT[:, 0:QBLK])
    sin_b = cs_pool.tile([128, QBLK], BF16, name="sin_0", tag="sin")
    nc.gpsimd.dma_start(out=sin_b, in_=sinS[:, 0:QBLK])
    cs_tiles[0] = (cos_b, sin_b)
    mD_sb = consts.tile([128, 128], BF16, name="mD_sb")
    nc.gpsimd.dma_start(out=mD_sb, in_=maskD[:, :])
    mT_sb = consts.tile([128, 128], BF16, name="mT_sb")
    nc.gpsimd.dma_start(out=mT_sb, in_=maskTl[:, :])

    wq_sb = consts.tile([128, NKT, O_SLICE], BF16, name="wq_sb")
    x_tiles = {}  # (qt, k) -> tile
    for k in range(NKT):
        nc.sync.dma_start(out=wq_sb[:, k, :], in_=wqkvT[k * 128:(k + 1) * 128, :])
        x_t = xt_pool.tile([128, QBLK], BF16, name=f"x_0_{k}", tag="xt")
        nc.sync.dma_start(out=x_t, in_=xT[k * 128:(k + 1) * 128, 0:QBLK])
        x_tiles[(0, k)] = x_t

    wp_sb = consts.tile([128, 4, C], BF16, name="wp_sb")  # loaded after A0
    # ones column carved out of the diag mask (col 127 <-> x=127 >= all lj);
    # f32r ones row for the normalize broadcast matmul
    ones_col = mD_sb[:, 127:128]             # [128, 1] bf16
    ones_row_f = consts.tile([1, 128], F32, name="ones_row_f")
    nc.vector.memset(ones_row_f, 1.0)
    ones_row = consts.tile([1, 128], F32R, name="ones_row")
    nc.vector.tensor_copy(ones_row, ones_row_f)
    ident = consts.tile([128, 128], BF16, name="ident")
    make_identity(nc, ident)
    # warm the ACT exp/tanh table set during the startup DMAs (first real
    # tanh would otherwise pay the ~1.3us ACT_TABLE_LOAD mid-pipeline)
    warmup = consts.tile([1, 1], F32, name="warmup")
    nc.scalar.activation(warmup, ident[0:1, 0:1],
                         mybir.ActivationFunctionType.Tanh)

    # persistent activations (written per block, sub-tile deps handle reuse)
    kT_sb = consts.tile([128, T], BF16, name="kT_sb")          # roped k, [d, t]
    v_sb = consts.tile([128, NKT, 128], BF16, name="v_sb")     # [t128, kt, d]
    qT_sb = consts.tile([128, 4, QBLK], BF16, name="qT_sb")    # roped q, [d, h, i]
    y_tiles = {}  # qt -> [128, 4, QBLK] tile, bufs=2 across blocks

    def emit_loads(qt):
        t0 = qt * QBLK
        for k in range(NKT):
            x_t = xt_pool.tile([128, QBLK], BF16, name=f"x_{qt}_{k}", tag="xt")
            nc.sync.dma_start(out=x_t, in_=xT[k * 128:(k + 1) * 128, t0:t0 + QBLK])
            x_tiles[(qt, k)] = x_t
        cos_b = cs_pool.tile([128, QBLK], BF16, name=f"cos_{qt}", tag="cos")
        nc.gpsimd.dma_start(out=cos_b, in_=cosT[:, t0:t0 + QBLK])
        sin_b = cs_pool.tile([128, QBLK], BF16, name=f"sin_{qt}", tag="sin")
        nc.gpsimd.dma_start(out=sin_b, in_=sinS[:, t0:t0 + QBLK])
        cs_tiles[qt] = (cos_b, sin_b)

    def emit_rope(qt, m, psA):
        t0 = qt * QBLK
        cos_b, sin_b = cs_tiles[qt]
        # block 0: PSUM->SBUF copies on the idle ACT queue so the six
        # back-to-back startup ropes pipeline instead of serializing on DVE;
        # sin-multiply always on Pool to balance DVE
        copy = nc.scalar.copy if qt == 0 else nc.vector.tensor_copy
        if m < 5:
            # RoPE: dest = x*cos + rot(x)*sin_signed ; rot via DMA half-swap
            x_sb = rope_pool.tile([128, QBLK], BF16, name=f"xsb_{qt}_{m}", tag="xsb")
            copy(x_sb, psA)
            rot = rope_pool.tile([128, QBLK], BF16, name=f"rot_{qt}_{m}", tag="rot")
            nc.gpsimd.dma_start(out=rot[0:64, :], in_=x_sb[64:128, :])
            nc.gpsimd.dma_start(out=rot[64:128, :], in_=x_sb[0:64, :])
            dest = qT_sb[:, m, :] if m < 4 else kT_sb[:, t0:t0 + QBLK]
            nc.vector.tensor_mul(x_sb, x_sb, cos_b)
            nc.vector.tensor_mul(rot, rot, sin_b)
            nc.vector.tensor_add(dest, x_sb, rot)
        else:
            # v: transpose [d, t] -> [t, d] tiles via PE (block 0's psT
            # borrows psA-tag banks freed by the m0/m1 ropes; later blocks
            # use psS as the scores stream frees those banks naturally)
            vt_sb = rope_pool.tile([128, QBLK], BF16, name=f"vt_{qt}", tag="xsb")
            copy(vt_sb, psA)
            for i in range(4):
                psT = ps.tile([128, 128], BF16, name=f"psT_{qt}_{i}",
                              tag="psA" if qt == 0 else "psS")
                nc.tensor.transpose(psT, vt_sb[:, i * 128:(i + 1) * 128], ident)
                nc.vector.tensor_copy(v_sb[:, qt * 4 + i, :], psT)

    def qkv_units(qt, m):
        """Fill units for one qkv m-tile: 8 x 2-matmul chunks + rope drain.
        Unit cost estimates are in ~us of PE time for the pop budget."""
        hold = {}

        def mk(i):
            def emit():
                if i == 0:
                    hold["psA"] = ps.tile([128, QBLK], F32,
                                          name=f"psA_{qt}_{m}", tag="psA")
                psA = hold["psA"]
                for k in (2 * i, 2 * i + 1):
                    nc.tensor.matmul(
                        psA,
                        wq_sb[:, k, m * 128:(m + 1) * 128],
                        x_tiles[(qt, k)],
                        start=(k == 0),
                        stop=(k == NKT - 1),
                    )
            return emit

        units = [(0.46, mk(i)) for i in range(8)]
        units.append((0.1, lambda: emit_rope(qt, m, hold["psA"])))
        return units

    # ---- startup: k-major qkv for block 0 (PE chases the DMA stream).
    # All six m-tiles accumulate simultaneously (6 of the 8 PSUM banks via
    # the psA/psS/psY tag pairs), so each arriving (wq_k, x_k) chunk yields
    # 6 matmuls immediately.  Ropes m2/m3 must precede m5: the v-transpose
    # PSUM tiles reuse the psS-tagged banks those accumulators hold. ----
    def startup_qkv():
        tags = ("psA", "psA", "psS", "psS", "psY", "psY")
        psQ = {m: ps.tile([128, QBLK], F32, name=f"psQ0_{m}", tag=tags[m],
                          bufs=2)
               for m in range(6)}
        for k in range(NKT):
            for m in range(6):
                nc.tensor.matmul(
                    psQ[m],
                    wq_sb[:, k, m * 128:(m + 1) * 128],
                    x_tiles[(0, k)],
                    start=(k == 0),
                    stop=(k == NKT - 1),
                    skip_group_check=True,
                )
        # kT first (scores dep), then q0/q1 (freeing the psA banks the
        # v-transposes borrow), v, then q2/q3
        for m in (4, 0, 1, 5, 2, 3):
            emit_rope(0, m, psQ[m])

    from collections import deque, defaultdict

    fill_q = deque()          # (cost, emit, key)
    pending = defaultdict(int)

    def queue_units(units, key=None):
        for cost, emit in units:
            fill_q.append((cost, emit, key))
            if key is not None:
                pending[key] += 1

    def pop_one():
        cost, emit, key = fill_q.popleft()
        emit()
        if key is not None:
            pending[key] -= 1
        return cost

    def pop_fill(budget):
        """Emit queued qkv/proj matmul units worth ~budget us of PE time —
        keeps the PE fed while the attention stream waits on ACT latency."""
        spent = 0.0
        while fill_q and spent < budget:
            spent += pop_one()

    def drain_until(key):
        """Force-emit every queued unit up to and including those for key
        (FIFO) so tiles the next head reads are defined before use."""
        while pending.get(key, 0) > 0:
            pop_one()

    def head_mms(qt, h, wl):
        """Scores/tanh-exp/mask/pv/rowsum matmul stream for one head.
        Scores are emitted one j-tile ahead of the pv/rowsum consumers, and
        queued qkv/proj fill units are popped between them, so the PE stays
        busy while ACT works through the tanh+exp latency."""
        psY = ps.tile([128, QBLK], F32, name=f"psY_{qt}_{h}", tag="psY", bufs=2)
        psB = ps.tile([128, QBLK], F32, name=f"psB_{qt}_{h}", tag="psB", bufs=2)

        def emit_scores(idx):
            kt, lo, hi, mk = wl[idx]
            psS = ps.tile([128, QBLK], F32, name=f"psS_{qt}_{h}_{kt}", tag="psS")
            nc.tensor.matmul(
                psS[:, lo:hi], kT_sb[:, kt * 128:(kt + 1) * 128],
                qT_sb[:, h, lo:hi],
                start=True, stop=True,
            )
            p_t = p_pool.tile([128, QBLK], BF16, name=f"p_{qt}_{h}_{kt}", tag="p")
            nc.scalar.activation(
                p_t[:, lo:hi], psS[:, lo:hi],
                mybir.ActivationFunctionType.Tanh, scale=scale1
            )
            nc.scalar.activation(
                p_t[:, lo:hi], p_t[:, lo:hi],
                mybir.ActivationFunctionType.Exp, scale=SOFTCAP
            )
            if mk is not None:
                kind, r = mk
                msk = mD_sb if kind == "D" else mT_sb
                nc.vector.tensor_mul(p_t[:, 128 * r:128 * r + 128],
                                     p_t[:, 128 * r:128 * r + 128], msk)
            return p_t

        pts = {0: emit_scores(0)}
        for idx, (kt, lo, hi, mk) in enumerate(wl):
            if idx + 1 < len(wl):
                pts[idx + 1] = emit_scores(idx + 1)
            pop_fill(0.85)
            p_t = pts.pop(idx)
            first, last = idx == 0, idx == len(wl) - 1
            nc.tensor.matmul(
                psY[:, lo:hi], v_sb[:, kt, :], p_t[:, lo:hi],
                start=first, stop=last, skip_group_check=True,
            )
            nc.tensor.matmul(
                psB[0:1, lo:hi], ones_col, p_t[:, lo:hi],
                start=first, stop=last, skip_group_check=True,
            )
        return psY, psB

    def norm_head(qt, h, psY, psB):
        """Stage A: free both PSUM accumulators fast with copies, then build
        1/rowsum as [1,512] via a [128,4] reshape (DVE recip is ~6
        cyc/elem/lane; [1,512] would serialize 3.3us).  The returned
        finisher broadcasts it across partitions with a ones-column matmul
        into psB (overwriting the rowsum row) and multiplies — it is
        deferred into the NEXT head's j-loop so the PE never waits on the
        reshape/recip chain."""
        rs = r_pool.tile([1, QBLK], F32, name=f"rs_{qt}_{h}", tag="rs")
        nc.vector.tensor_copy(rs, psB[0:1, :])
        yun = r_pool.tile([128, QBLK], F32, name=f"yun_{qt}_{h}", tag="yun")
        nc.vector.tensor_copy(yun, psY)
        rs128 = r_pool.tile([128, 4], F32, name=f"rs128_{qt}_{h}", tag="rs128")
        in_lin = bass.AP(tensor=rs.tensor, offset=rs.offset,
                         ap=[list(rs.ap[0]), [1, QBLK]])
        nc.gpsimd.dma_start(out=rs128, in_=in_lin)
        rr128 = r_pool.tile([128, 4], F32, name=f"rr128_{qt}_{h}", tag="rr128")
        nc.vector.reciprocal(rr128, rs128)
        rr_row = r_pool.tile([1, QBLK], F32R, name=f"rrw_{qt}_{h}", tag="rrw")
        row_view = bass.AP(tensor=rr_row.tensor, offset=rr_row.offset,
                           ap=[list(rr_row.ap[0]), [1, QBLK]])
        nc.gpsimd.dma_start(out=row_view, in_=rr128)

        def finish():
            nc.tensor.matmul(psB, ones_row, rr_row, start=True, stop=True,
                             skip_group_check=True)
            nc.vector.tensor_mul(y_tiles[qt][:, h, :], yun, psB)
        return finish

    def proj_units(qt, mt):
        t0 = qt * QBLK

        def mk(cn):
            def emit():
                psP = ps.tile([128, 512], F32,
                              name=f"psP_{qt}_{mt}_{cn}", tag="psA")
                yt = y_tiles[qt]
                for kh in range(4):
                    nc.tensor.matmul(
                        psP,
                        yt[:, kh, mt * 128:(mt + 1) * 128],
                        wp_sb[:, kh, cn * 512:(cn + 1) * 512],
                        start=(kh == 0),
                        stop=(kh == 3),
                    )
                o_t = o_pool.tile([128, 512], F32,
                                  name=f"o_{qt}_{mt}_{cn}", tag="o")
                nc.vector.tensor_copy(o_t, psP)
                nc.sync.dma_start(
                    out=out[t0 + mt * 128: t0 + (mt + 1) * 128,
                            cn * 512:(cn + 1) * 512],
                    in_=o_t,
                )
            return emit

        return [(0.9, mk(cn)) for cn in range(4)]

    def emit_proj_chunk(qt, mt):
        for _, emit in proj_units(qt, mt):
            emit()

    finz = {}  # (qt, h) -> deferred normalize finisher

    def push_fin(qt, h):
        f = finz.pop((qt, h), None)
        if f is not None:
            fill_q.appendleft((0.25, f, None))

    # ---- interleaved pipeline with fine-grained fills ----
    startup_qkv()
    for qt in range(NQB):
        if qt + 1 < NQB:
            emit_loads(qt + 1)
        wl = _window(qt)
        y_tiles[qt] = consts.tile([128, 4, QBLK], BF16,
                                  name=f"yT_{qt}", tag="yT", bufs=2)
        for h in range(4):
            # finishers ride the FRONT of the fill queue two heads after
            # their stage A, so the PE never waits on the recip chain but
            # y[h] is always emitted before any proj fill that reads it
            if h == 0 and qt >= 1:
                push_fin(qt - 1, 3)
                push_fin(qt - 1, 2)
            elif h >= 2:
                push_fin(qt, h - 2)
            # qkv fills for the next block (block 3's m1/m2/m3 are held
            # back and queued during block 3's own attention)
            if qt + 1 < NQB:
                nxt = qt + 1
                if nxt < 3:
                    queue_units(qkv_units(nxt, (4, 5, 0, 1)[h]),
                                key=("q", nxt, (4, 5, 0, 1)[h]))
                elif h < 3:
                    queue_units(qkv_units(3, (4, 5, 0)[h]),
                                key=("q", 3, (4, 5, 0)[h]))
            else:
                if h < 3:
                    queue_units(qkv_units(3, h + 1), key=("q", 3, h + 1))
            # proj fills for the previous block (shifted one head later in
            # the last block so DVE runs the final normalizes promptly)
            if qt >= 1:
                if qt == NQB - 1:
                    if h >= 1:
                        queue_units(proj_units(qt - 1, h - 1))
                else:
                    queue_units(proj_units(qt - 1, h))
            # def-before-use: everything this head reads must be emitted
            if qt >= 1:
                drain_until(("q", qt, 4))
                drain_until(("q", qt, 5))
                drain_until(("q", qt, h))
            psY, psB = head_mms(qt, h, wl)
            finz[(qt, h)] = norm_head(qt, h, psY, psB)
        if qt == 0:
            # cn-major chunks so proj(0, *, cn) deps resolve incrementally
            for cn in range(4):
                for kh in range(4):
                    nc.sync.dma_start(
                        out=wp_sb[:, kh, cn * 512:(cn + 1) * 512],
                        in_=wprojT[kh * 128:(kh + 1) * 128,
                                   cn * 512:(cn + 1) * 512])
        if qt + 1 < 3:
            queue_units(qkv_units(qt + 1, 2), key=("q", qt + 1, 2))
            queue_units(qkv_units(qt + 1, 3), key=("q", qt + 1, 3))
    queue_units(proj_units(NQB - 2, 3))
    while fill_q:
        pop_one()
    finz.pop((NQB - 1, 2))()
    finz.pop((NQB - 1, 3))()
    for mt in range(4):
        emit_proj_chunk(NQB - 1, mt)

_NC_CACHE = {}


def _build_nc():
    if "nc" not in _NC_CACHE:
        from contextlib import ExitStack

        from concourse import bacc

        nc = bacc.Bacc()
        with tile.TileContext(nc) as tc, ExitStack() as ctx:
            _emit(tc, ctx)
        nc.compile()
        _NC_CACHE["nc"] = nc
    return _NC_CACHE["nc"]


def _host_inputs(x, cos, sin, Wqkv, Wproj):
    """Build the 8 per-core input maps (sharding + layout transforms)."""
    bf16 = ml_dtypes.bfloat16
    cosT = np.ascontiguousarray(cos.T).astype(bf16)          # [128, T]
    sinT = sin.T
    sinS = np.concatenate([-sinT[:64], sinT[64:]], axis=0)   # sign-folded rotate-half
    sinS = np.ascontiguousarray(sinS).astype(bf16)
    lj = np.arange(128)[:, None]
    xs = np.arange(128)[None, :]                             # x = li - 128r in [0,128)
    maskD = (xs >= lj).astype(bf16)                          # diag: li - 128r >= lj
    maskTl = (xs < lj).astype(bf16)                          # tail: li - 128r <  lj
    q_sz = N_HEAD * HEAD_SIZE

    in_maps = []
    for core in range(8):
        b, g = core // 4, core % 4
        xTb = np.ascontiguousarray(x[b].T).astype(bf16)      # [C, T]
        wslice = np.concatenate(
            [
                Wqkv[512 * g: 512 * (g + 1)],                 # 4 q heads
                Wqkv[q_sz + 128 * g: q_sz + 128 * (g + 1)],   # k
                Wqkv[q_sz + 512 + 128 * g: q_sz + 512 + 128 * (g + 1)],  # v
            ],
            axis=0,
        )                                                     # [768, C]
        wqkvT = np.ascontiguousarray(wslice.T).astype(bf16)   # [C, 768]
        wprojT = np.ascontiguousarray(
            Wproj[:, 512 * g: 512 * (g + 1)].T).astype(bf16)  # [512, C]
        in_maps.append(
            {
                "xT": xTb,
                "wqkvT": wqkvT,
                "wprojT": wprojT,
                "cosT": cosT,
                "sinS": sinS,
                "maskD": maskD,
                "maskTl": maskTl,
            }
        )
    return in_maps


def kernel(x, cos, sin, Wqkv, Wproj, trace=False, tmpdir=None):
    x = np.asarray(x, dtype=np.float32)
    cos = np.asarray(cos, dtype=np.float32)
    sin = np.asarray(sin, dtype=np.float32)
    Wqkv = np.asarray(Wqkv, dtype=np.float32)
    Wproj = np.asarray(Wproj, dtype=np.float32)

    nc = _build_nc()
    in_maps = _host_inputs(x, cos, sin, Wqkv, Wproj)
    res = run_bass_kernel_spmd(nc, in_maps, list(range(8)), trace=trace, tmpdir=tmpdir)
    parts = [res.results[i]["out"] for i in range(8)]
    out = np.empty((B, T, C), dtype=np.float32)
    out[0] = parts[0] + parts[1] + parts[2] + parts[3]
    out[1] = parts[4] + parts[5] + parts[6] + parts[7]
    if trace:
        kernel.last_exec_time_ns = res.exec_time_ns
        kernel.last_results = res
    return out


# revision 28
# speedup vs baseline: 1.2519x; 1.0010x over previous
"""Sliding-window causal self-attention (GQA + RoPE + tanh softcap) on 8 trn2 cores.

Sharding: core i = (b, g) with b = i // 4, g = i % 4.
Each core computes, for its batch b and kv-group g (4 q heads, 1 kv head):
    qkv projection (o-slice), RoPE, sliding-window attention, and the proj
    contribution of its o-slice:  out_partial[t, c] = sum_{o in slice} y[t,o] Wproj[c,o].
The host sums the 4 partials per batch (the "all-reduce after proj" done host-side).

All matmuls run as fp32r (full PE rate at N>=256 when warm). Layouts avoid
on-device transposes except v (PE-transpose via identity):
    xT      [C, T]   (host-transposed x[b])
    wqkvT   [C, 768] (host-transposed o-slice of Wqkv; o order: q0..q3, k, v)
    wprojT  [512, C] (host-transposed o-slice columns of Wproj)
    qT/kT   [d, t]   from  wT.T @ xT  (d on partitions -> scoresT = kT_tile.T @ qT)
    scoresT [j, i]   j (keys) on partitions, i (queries) on free axis
    P = exp(softcap(scores)) stays [j, i]; yT = v_tile.T @ P accumulates [d, i]
    rowsum via ones-matmul into psB row 0; normalize = [128,4]-reshaped DVE
    recip, ones-row PE matmul broadcasting 1/rowsum across psB, DVE mult.

Startup is HBM-bound (wqkv 6MB + x0 4MB stream at ~400GB/s): block-0 qkv is
emitted K-MAJOR over three open PSUM accumulators per phase (m 0-2, then
m 3-5) so each arriving (wq_k, x_k) chunk immediately yields 3 matmuls and
the PE tracks the DMA stream instead of stalling m-major.

Masked score tiles are NARROWED to their live column span (>=256 so fp32r
keeps 1 cyc/row): diag tile r spans [min(128r,256), 512), tail tile r spans
[0, max(128(r+1),256)). Scores/tanh/exp/PV/rowsum all honor the span, and
the 0/1 mask multiply only touches the 128-col staircase block (256 for
D3/T0 which carry a dead quarter). Window order puts a full-span tile first
so PSUM start=True covers every column.

Block pipeline: qkv m-tiles of block qt+1 and proj chunks of block qt-1 are
queued as small matmul units and drained INSIDE the attention j-loops, so
the PE stays busy while ACT works through tanh+exp latency. Block 3 (which
has no next-qkv) holds back qkv(3) m1/m2/m3 into its own attention phase,
with drain guards that force any unit producing qT/kT/v of head h to be
emitted before head h's scores.
"""

import math

import ml_dtypes
import numpy as np

import concourse.bass as bass
import concourse.mybir as mybir
import concourse.tile as tile
from concourse.bass_utils import run_bass_kernel_spmd
from concourse.masks import make_identity

B, T, C = 2, 2048, 2048
N_HEAD, N_GROUPS, HEAD_SIZE = 16, 4, 128
SW = 1024
SOFTCAP = 50.0
QBLK = 512
NQB = T // QBLK          # 4 q-blocks
NKT = T // 128           # 16 key tiles
O_SLICE = 768            # 4 q heads + k + v  (128 each)
F32 = mybir.dt.float32
F32R = mybir.dt.float32r
BF16 = mybir.dt.bfloat16


def _window(qt):
    """Key-tile list for q-block qt: (kt, lo, hi, mask) with mask
    None | ('D', r) | ('T', r).  [lo, hi) is the live query-column span
    (clamped to >=256 wide for full-rate fp32r).  A full-span tile is
    always first so the PSUM start=True write covers all 512 columns."""
    wl = []
    for kt in range(max(0, 4 * qt - 4), 4 * qt):
        wl.append((kt, 0, 512, None))
    if qt >= 2:
        for r in range(4):
            wl.append((4 * qt - 8 + r, 0, 128 * (r + 1), ("T", r)))
    for r in range(4):
        wl.append((4 * qt + r, 128 * r, 512, ("D", r)))
    if qt == 0:
        # D0 has full span; it must lead for start=True coverage
        assert wl[0][3] == ("D", 0) and wl[0][1] == 0 and wl[0][2] == 512
    return wl


def _emit(tc, ctx):
    nc = tc.nc
    xT = nc.declare_dram_parameter("xT", [C, T], BF16, isOutput=False)
    wqkvT = nc.declare_dram_parameter("wqkvT", [C, O_SLICE], BF16, isOutput=False)
    wprojT = nc.declare_dram_parameter("wprojT", [512, C], BF16, isOutput=False)
    cosT = nc.declare_dram_parameter("cosT", [HEAD_SIZE, T], BF16, isOutput=False)
    sinS = nc.declare_dram_parameter("sinS", [HEAD_SIZE, T], BF16, isOutput=False)
    maskD = nc.declare_dram_parameter("maskD", [128, 128], BF16, isOutput=False)
    maskTl = nc.declare_dram_parameter("maskTl", [128, 128], BF16, isOutput=False)
    out = nc.declare_dram_parameter("out", [T, C], F32, isOutput=True)

    scale1 = 1.0 / (SOFTCAP * math.sqrt(HEAD_SIZE))

    consts = ctx.enter_context(tc.tile_pool(name="consts", bufs=1))
    xt_pool = ctx.enter_context(tc.tile_pool(name="xt", bufs=16))
    cs_pool = ctx.enter_context(tc.tile_pool(name="cs", bufs=2))
    rope_pool = ctx.enter_context(tc.tile_pool(name="rope", bufs=3))
    p_pool = ctx.enter_context(tc.tile_pool(name="pp", bufs=5))
    o_pool = ctx.enter_context(tc.tile_pool(name="op", bufs=3))
    r_pool = ctx.enter_context(tc.tile_pool(name="rp", bufs=2))
    ps = ctx.enter_context(tc.tile_pool(name="ps", space="PSUM", bufs=2))

    # ---- small constants (cos/sin/masks) go FIRST on the idle GpSimd
    # queue — behind the Sync stream they'd land ~45us in and stall the
    # block-0 ropes.  The big wq+x0 stream rides Sync as interleaved
    # (wq_k, x_k) pairs: one DMA instruction rides one DMA engine, so many
    # mid-size DMAs beat few large ones (transfer-bound ~400GB/s). ----
    cs_tiles = {}
    cos_b = cs_pool.tile([128, QBLK], BF16, name="cos_0", tag="cos")
    nc.gpsimd.dma_start(out=cos_b, in_=cosT[:, 0:QBLK])
    sin_b = cs_pool.tile([128, QBLK], BF16, name="sin_0", tag="sin")
    nc.gpsimd.dma_start(out=sin_b, in_=sinS[:, 0:QBLK])
    cs_tiles[0] = (cos_b, sin_b)
    mD_sb = consts.tile([128, 128], BF16, name="mD_sb")
    nc.gpsimd.dma_start(out=mD_sb, in_=maskD[:, :])
    mT_sb = consts.tile([128, 128], BF16, name="mT_sb")
    nc.gpsimd.dma_start(out=mT_sb, in_=maskTl[:, :])

    wq_sb = consts.tile([128, NKT, O_SLICE], BF16, name="wq_sb")
    x_tiles = {}  # (qt, k) -> tile
    for k in range(NKT):
        nc.sync.dma_start(out=wq_sb[:, k, :], in_=wqkvT[k * 128:(k + 1) * 128, :])
        x_t = xt_pool.tile([128, QBLK], BF16, name=f"x_0_{k}", tag="xt")
        nc.sync.dma_start(out=x_t, in_=xT[k * 128:(k + 1) * 128, 0:QBLK])
        x_tiles[(0, k)] = x_t

    wp_sb = consts.tile([128, 4, C], BF16, name="wp_sb")  # loaded after A0
    # ones column carved out of the diag mask (col 127 <-> x=127 >= all lj);
    # f32r ones row for the normalize broadcast matmul
    ones_col = mD_sb[:, 127:128]             # [128, 1] bf16
    ones_row_f = consts.tile([1, 128], F32, name="ones_row_f")
    nc.vector.memset(ones_row_f, 1.0)
    ones_row = consts.tile([1, 128], F32R, name="ones_row")
    nc.vector.tensor_copy(ones_row, ones_row_f)
    ident = consts.tile([128, 128], BF16, name="ident")
    make_identity(nc, ident)
    # warm the ACT exp/tanh table set during the startup DMAs (first real
    # tanh would otherwise pay the ~1.3us ACT_TABLE_LOAD mid-pipeline)
    warmup = consts.tile([1, 1], F32, name="warmup")
    nc.scalar.activation(warmup, ident[0:1, 0:1],
                         mybir.ActivationFunctionType.Tanh)

    # persistent activations (written per block, sub-tile deps handle reuse)
    kT_sb = consts.tile([128, T], BF16, name="kT_sb")          # roped k, [d, t]
    v_sb = consts.tile([128, NKT, 128], BF16, name="v_sb")     # [t128, kt, d]
    qT_sb = consts.tile([128, 4, QBLK], BF16, name="qT_sb")    # roped q, [d, h, i]
    y_tiles = {}  # qt -> [128, 4, QBLK] tile, bufs=2 across blocks

    def emit_loads(qt):
        t0 = qt * QBLK
        for k in range(NKT):
            x_t = xt_pool.tile([128, QBLK], BF16, name=f"x_{qt}_{k}", tag="xt")
            nc.sync.dma_start(out=x_t, in_=xT[k * 128:(k + 1) * 128, t0:t0 + QBLK])
            x_tiles[(qt, k)] = x_t
        cos_b = cs_pool.tile([128, QBLK], BF16, name=f"cos_{qt}", tag="cos")
        nc.gpsimd.dma_start(out=cos_b, in_=cosT[:, t0:t0 + QBLK])
        sin_b = cs_pool.tile([128, QBLK], BF16, name=f"sin_{qt}", tag="sin")
        nc.gpsimd.dma_start(out=sin_b, in_=sinS[:, t0:t0 + QBLK])
        cs_tiles[qt] = (cos_b, sin_b)

    def emit_rope(qt, m, psA):
        t0 = qt * QBLK
        cos_b, sin_b = cs_tiles[qt]
        # block 0: PSUM->SBUF copies on the idle ACT queue so the six
        # back-to-back startup ropes pipeline instead of serializing on DVE;
        # sin-multiply always on Pool to balance DVE
        copy = nc.scalar.copy if qt == 0 else nc.vector.tensor_copy
        if m < 5:
            # RoPE: dest = x*cos + rot(x)*sin_signed ; rot via DMA half-swap
            x_sb = rope_pool.tile([128, QBLK], BF16, name=f"xsb_{qt}_{m}", tag="xsb")
            copy(x_sb, psA)
            rot = rope_pool.tile([128, QBLK], BF16, name=f"rot_{qt}_{m}", tag="rot")
            nc.gpsimd.dma_start(out=rot[0:64, :], in_=x_sb[64:128, :])
            nc.gpsimd.dma_start(out=rot[64:128, :], in_=x_sb[0:64, :])
            dest = qT_sb[:, m, :] if m < 4 else kT_sb[:, t0:t0 + QBLK]
            nc.vector.tensor_mul(x_sb, x_sb, cos_b)
            nc.vector.tensor_mul(rot, rot, sin_b)
            nc.vector.tensor_add(dest, x_sb, rot)
        else:
            # v: transpose [d, t] -> [t, d] tiles via PE (block 0's psT
            # borrows psA-tag banks freed by the m0/m1 ropes; later blocks
            # use psS as the scores stream frees those banks naturally)
            vt_sb = rope_pool.tile([128, QBLK], BF16, name=f"vt_{qt}", tag="xsb")
            copy(vt_sb, psA)
            for i in range(4):
                psT = ps.tile([128, 128], BF16, name=f"psT_{qt}_{i}",
                              tag="psA" if qt == 0 else "psS")
                nc.tensor.transpose(psT, vt_sb[:, i * 128:(i + 1) * 128], ident)
                nc.vector.tensor_copy(v_sb[:, qt * 4 + i, :], psT)

    def qkv_units(qt, m):
        """Fill units for one qkv m-tile: 8 x 2-matmul chunks + rope drain.
        Unit cost estimates are in ~us of PE time for the pop budget."""
        hold = {}

        def mk(i):
            def emit():
                if i == 0:
                    hold["psA"] = ps.tile([128, QBLK], F32,
                                          name=f"psA_{qt}_{m}", tag="psA")
                psA = hold["psA"]
                for k in (2 * i, 2 * i + 1):
                    nc.tensor.matmul(
                        psA,
                        wq_sb[:, k, m * 128:(m + 1) * 128],
                        x_tiles[(qt, k)],
                        start=(k == 0),
                        stop=(k == NKT - 1),
                    )
            return emit

        units = [(0.46, mk(i)) for i in range(8)]
        units.append((0.1, lambda: emit_rope(qt, m, hold["psA"])))
        return units

    # ---- startup: k-major qkv for block 0 (PE chases the DMA stream).
    # All six m-tiles accumulate simultaneously (6 of the 8 PSUM banks via
    # the psA/psS/psY tag pairs), so each arriving (wq_k, x_k) chunk yields
    # 6 matmuls immediately.  Ropes m2/m3 must precede m5: the v-transpose
    # PSUM tiles reuse the psS-tagged banks those accumulators hold. ----
    def startup_qkv():
        tags = ("psA", "psA", "psS", "psS", "psY", "psY")
        psQ = {m: ps.tile([128, QBLK], F32, name=f"psQ0_{m}", tag=tags[m],
                          bufs=2)
               for m in range(6)}
        for k in range(NKT):
            for m in range(6):
                nc.tensor.matmul(
                    psQ[m],
                    wq_sb[:, k, m * 128:(m + 1) * 128],
                    x_tiles[(0, k)],
                    start=(k == 0),
                    stop=(k == NKT - 1),
                    skip_group_check=True,
                )
        # kT first (scores dep), then q0/q1 (freeing the psA banks the
        # v-transposes borrow), v, then q2/q3
        for m in (4, 0, 1, 5, 2, 3):
            emit_rope(0, m, psQ[m])

    from collections import deque, defaultdict

    fill_q = deque()          # (cost, emit, key)
    pending = defaultdict(int)

    def queue_units(units, key=None):
        for cost, emit in units:
            fill_q.append((cost, emit, key))
            if key is not None:
                pending[key] += 1

    def pop_one():
        cost, emit, key = fill_q.popleft()
        emit()
        if key is not None:
            pending[key] -= 1
        return cost

    def pop_fill(budget):
        """Emit queued qkv/proj matmul units worth ~budget us of PE time —
        keeps the PE fed while the attention stream waits on ACT latency."""
        spent = 0.0
        while fill_q and spent < budget:
            spent += pop_one()

    def drain_until(key):
        """Force-emit every queued unit up to and including those for key
        (FIFO) so tiles the next head reads are defined before use."""
        while pending.get(key, 0) > 0:
            pop_one()

    def head_mms(qt, h, wl):
        """Scores/tanh-exp/mask/pv/rowsum matmul stream for one head.
        Scores are emitted one j-tile ahead of the pv/rowsum consumers, and
        queued qkv/proj fill units are popped between them, so the PE stays
        busy while ACT works through the tanh+exp latency."""
        psY = ps.tile([128, QBLK], F32, name=f"psY_{qt}_{h}", tag="psY", bufs=2)
        psB = ps.tile([128, QBLK], F32, name=f"psB_{qt}_{h}", tag="psB", bufs=2)

        def emit_scores(idx):
            kt, lo, hi, mk = wl[idx]
            psS = ps.tile([128, QBLK], F32, name=f"psS_{qt}_{h}_{kt}", tag="psS")
            nc.tensor.matmul(
                psS[:, lo:hi], kT_sb[:, kt * 128:(kt + 1) * 128],
                qT_sb[:, h, lo:hi],
                start=True, stop=True,
            )
            p_t = p_pool.tile([128, QBLK], BF16, name=f"p_{qt}_{h}_{kt}", tag="p")
            nc.scalar.activation(
                p_t[:, lo:hi], psS[:, lo:hi],
                mybir.ActivationFunctionType.Tanh, scale=scale1
            )
            nc.scalar.activation(
                p_t[:, lo:hi], p_t[:, lo:hi],
                mybir.ActivationFunctionType.Exp, scale=SOFTCAP
            )
            if mk is not None:
                kind, r = mk
                msk = mD_sb if kind == "D" else mT_sb
                nc.vector.tensor_mul(p_t[:, 128 * r:128 * r + 128],
                                     p_t[:, 128 * r:128 * r + 128], msk)
            return p_t

        pts = {0: emit_scores(0)}
        for idx, (kt, lo, hi, mk) in enumerate(wl):
            if idx + 1 < len(wl):
                pts[idx + 1] = emit_scores(idx + 1)
            pop_fill(0.85)
            p_t = pts.pop(idx)
            first, last = idx == 0, idx == len(wl) - 1
            nc.tensor.matmul(
                psY[:, lo:hi], v_sb[:, kt, :], p_t[:, lo:hi],
                start=first, stop=last, skip_group_check=True,
            )
            nc.tensor.matmul(
                psB[0:1, lo:hi], ones_col, p_t[:, lo:hi],
                start=first, stop=last, skip_group_check=True,
            )
        return psY, psB

    def norm_head(qt, h, psY, psB):
        """Stage A: free both PSUM accumulators fast with copies, then
        1/rowsum directly on the [1,512] row via reciprocal_approx_fast
        (~18-bit, plenty for a softmax denominator; rowsum is in
        [e^-50, 1024 e^50] so no edge cases).  The returned finisher
        broadcasts it across partitions with a ones-row matmul into psB
        (overwriting the rowsum row) and multiplies — it is deferred into
        the j-loop two heads later so the PE never waits on the chain."""
        rs = r_pool.tile([1, QBLK], F32, name=f"rs_{qt}_{h}", tag="rs")
        nc.vector.tensor_copy(rs, psB[0:1, :])
        yun = r_pool.tile([128, QBLK], F32, name=f"yun_{qt}_{h}", tag="yun")
        nc.vector.tensor_copy(yun, psY)
        rr_f = r_pool.tile([1, QBLK], F32, name=f"rrf_{qt}_{h}", tag="rrf")
        nc.vector.reciprocal_approx_fast(out=rr_f, in_=rs)
        rr_row = r_pool.tile([1, QBLK], F32R, name=f"rrw_{qt}_{h}", tag="rrw")
        nc.vector.tensor_copy(rr_row, rr_f)

        def finish():
            nc.tensor.matmul(psB, ones_row, rr_row, start=True, stop=True,
                             skip_group_check=True)
            nc.vector.tensor_mul(y_tiles[qt][:, h, :], yun, psB)
        return finish

    def proj_units(qt, mt):
        t0 = qt * QBLK

        def mk(cn):
            def emit():
                psP = ps.tile([128, 512], F32,
                              name=f"psP_{qt}_{mt}_{cn}", tag="psA")
                yt = y_tiles[qt]
                for kh in range(4):
                    nc.tensor.matmul(
                        psP,
                        yt[:, kh, mt * 128:(mt + 1) * 128],
                        wp_sb[:, kh, cn * 512:(cn + 1) * 512],
                        start=(kh == 0),
                        stop=(kh == 3),
                    )
                o_t = o_pool.tile([128, 512], F32,
                                  name=f"o_{qt}_{mt}_{cn}", tag="o")
                nc.vector.tensor_copy(o_t, psP)
                nc.sync.dma_start(
                    out=out[t0 + mt * 128: t0 + (mt + 1) * 128,
                            cn * 512:(cn + 1) * 512],
                    in_=o_t,
                )
            return emit

        return [(0.9, mk(cn)) for cn in range(4)]

    def emit_proj_chunk(qt, mt):
        for _, emit in proj_units(qt, mt):
            emit()

    finz = {}  # (qt, h) -> deferred normalize finisher

    def push_fin(qt, h):
        f = finz.pop((qt, h), None)
        if f is not None:
            fill_q.appendleft((0.25, f, None))

    # ---- interleaved pipeline with fine-grained fills ----
    startup_qkv()
    for qt in range(NQB):
        if qt + 1 < NQB:
            emit_loads(qt + 1)
        wl = _window(qt)
        y_tiles[qt] = consts.tile([128, 4, QBLK], BF16,
                                  name=f"yT_{qt}", tag="yT", bufs=2)
        for h in range(4):
            # finishers ride the FRONT of the fill queue two heads after
            # their stage A, so the PE never waits on the recip chain but
            # y[h] is always emitted before any proj fill that reads it
            if h == 0 and qt >= 1:
                push_fin(qt - 1, 3)
                push_fin(qt - 1, 2)
            elif h >= 2:
                push_fin(qt, h - 2)
            # qkv fills for the next block (block 3's m1/m2/m3 are held
            # back and queued during block 3's own attention)
            if qt + 1 < NQB:
                nxt = qt + 1
                if nxt < 3:
                    queue_units(qkv_units(nxt, (4, 5, 0, 1)[h]),
                                key=("q", nxt, (4, 5, 0, 1)[h]))
                elif h < 3:
                    queue_units(qkv_units(3, (4, 5, 0)[h]),
                                key=("q", 3, (4, 5, 0)[h]))
            else:
                if h < 3:
                    queue_units(qkv_units(3, h + 1), key=("q", 3, h + 1))
            # proj fills for the previous block (shifted one head later in
            # the last block so DVE runs the final normalizes promptly)
            if qt >= 1:
                if qt == NQB - 1:
                    if h >= 1:
                        queue_units(proj_units(qt - 1, h - 1))
                else:
                    queue_units(proj_units(qt - 1, h))
            # def-before-use: everything this head reads must be emitted
            if qt >= 1:
                drain_until(("q", qt, 4))
                drain_until(("q", qt, 5))
                drain_until(("q", qt, h))
            psY, psB = head_mms(qt, h, wl)
            finz[(qt, h)] = norm_head(qt, h, psY, psB)
        if qt == 0:
            # cn-major chunks so proj(0, *, cn) deps resolve incrementally
            for cn in range(4):
                for kh in range(4):
                    nc.sync.dma_start(
                        out=wp_sb[:, kh, cn * 512:(cn + 1) * 512],
                        in_=wprojT[kh * 128:(kh + 1) * 128,
                                   cn * 512:(cn + 1) * 512])
        if qt + 1 < 3:
            queue_units(qkv_units(qt + 1, 2), key=("q", qt + 1, 2))
            queue_units(qkv_units(qt + 1, 3), key=("q", qt + 1, 3))
    queue_units(proj_units(NQB - 2, 3))
    while fill_q:
        pop_one()
    finz.pop((NQB - 1, 2))()
    finz.pop((NQB - 1, 3))()
    for mt in range(4):
        emit_proj_chunk(NQB - 1, mt)

_NC_CACHE = {}


def _build_nc():
    if "nc" not in _NC_CACHE:
        from contextlib import ExitStack

        from concourse import bacc

        nc = bacc.Bacc()
        with tile.TileContext(nc) as tc, ExitStack() as ctx:
            _emit(tc, ctx)
        nc.compile()
        _NC_CACHE["nc"] = nc
    return _NC_CACHE["nc"]


def _host_inputs(x, cos, sin, Wqkv, Wproj):
    """Build the 8 per-core input maps (sharding + layout transforms)."""
    bf16 = ml_dtypes.bfloat16
    cosT = np.ascontiguousarray(cos.T).astype(bf16)          # [128, T]
    sinT = sin.T
    sinS = np.concatenate([-sinT[:64], sinT[64:]], axis=0)   # sign-folded rotate-half
    sinS = np.ascontiguousarray(sinS).astype(bf16)
    lj = np.arange(128)[:, None]
    xs = np.arange(128)[None, :]                             # x = li - 128r in [0,128)
    maskD = (xs >= lj).astype(bf16)                          # diag: li - 128r >= lj
    maskTl = (xs < lj).astype(bf16)                          # tail: li - 128r <  lj
    q_sz = N_HEAD * HEAD_SIZE

    in_maps = []
    for core in range(8):
        b, g = core // 4, core % 4
        xTb = np.ascontiguousarray(x[b].T).astype(bf16)      # [C, T]
        wslice = np.concatenate(
            [
                Wqkv[512 * g: 512 * (g + 1)],                 # 4 q heads
                Wqkv[q_sz + 128 * g: q_sz + 128 * (g + 1)],   # k
                Wqkv[q_sz + 512 + 128 * g: q_sz + 512 + 128 * (g + 1)],  # v
            ],
            axis=0,
        )                                                     # [768, C]
        wqkvT = np.ascontiguousarray(wslice.T).astype(bf16)   # [C, 768]
        wprojT = np.ascontiguousarray(
            Wproj[:, 512 * g: 512 * (g + 1)].T).astype(bf16)  # [512, C]
        in_maps.append(
            {
                "xT": xTb,
                "wqkvT": wqkvT,
                "wprojT": wprojT,
                "cosT": cosT,
                "sinS": sinS,
                "maskD": maskD,
                "maskTl": maskTl,
            }
        )
    return in_maps


def kernel(x, cos, sin, Wqkv, Wproj, trace=False, tmpdir=None):
    x = np.asarray(x, dtype=np.float32)
    cos = np.asarray(cos, dtype=np.float32)
    sin = np.asarray(sin, dtype=np.float32)
    Wqkv = np.asarray(Wqkv, dtype=np.float32)
    Wproj = np.asarray(Wproj, dtype=np.float32)

    nc = _build_nc()
    in_maps = _host_inputs(x, cos, sin, Wqkv, Wproj)
    res = run_bass_kernel_spmd(nc, in_maps, list(range(8)), trace=trace, tmpdir=tmpdir)
    parts = [res.results[i]["out"] for i in range(8)]
    out = np.empty((B, T, C), dtype=np.float32)
    out[0] = parts[0] + parts[1] + parts[2] + parts[3]
    out[1] = parts[4] + parts[5] + parts[6] + parts[7]
    if trace:
        kernel.last_exec_time_ns = res.exec_time_ns
        kernel.last_results = res
    return out
